# revision 1
# baseline (speedup 1.0000x reference)
"""Trainium2 8-core kernel for batched attention + concat projection.

Reference computation (per batch b):
    scores = Q @ C^T                  [TQ, TC]
    A      = softmax(scores, axis=-1)
    mix    = A @ C                    [TQ, H]
    out    = tanh(concat([mix, Q]) @ W^T)   [TQ, H]

Distribution: pure data-parallel over batch (B=16 across 8 cores, 2
batches per core), W replicated. No collectives needed.

Per-core dataflow (activations kept in "transposed" [feature, token]
layout so every matmul contracts over the partition axis):
  - CT = C^T (f32r) and QT = Q^T built on-device via f32r PE
    transposes (1.5 cycles/col vs 2.0 for plain f32).
  - scores S[q,k] = QT.T @ CT  (f32r matmuls, 1 col/cycle).
  - softmax over free axis k: DVE reduce_max(negate) -> ACT exp with
    per-partition bias, bf16 output (unnormalized, max ~= 1) and
    fp32 row-sum accumulator -> DVE reciprocal.
  - P^T via bf16 PE transposes; the PSUM drains cast each k-half to
    its PV dtype.  mix^T = C.T @ P^T with mixed-precision k-halves:
    fp8e4 DoubleRow (2 k-tiles/pass, 2x rate) where allowed by the
    error budget, bf16 elsewhere (see hi_fp8 below).
  - normalization folded into the PV PSUM drain: multiply by a
    [128, sq] broadcast of 1/rowsum built once per super-iteration on
    the PE (transpose rcp to a row + ones outer-product matmul).
  - proj: out[q, :] = tanh(combT.T @ W^T) in bf16, W^T pre-transposed
    on host.

The P^T/PV/proj stages for super-iteration s are emitted one
super-iteration later (software pipelining) so the in-order TensorE
stream always has ready matmul work while the softmax chain of the
current tile runs on ACT/DVE.
"""

import numpy as np
import ml_dtypes

import concourse.bacc as bacc
import concourse.tile as tile
import concourse.mybir as mybir
from concourse.bass_utils import run_bass_kernel_spmd

F32 = mybir.dt.float32
F32R = mybir.dt.float32r
BF16 = mybir.dt.bfloat16
FP8 = mybir.dt.float8e4

N_CORES = 8
B, TQ, TC, H = 16, 2048, 2048, 1024

# fp8 DoubleRow PV runs at 2x the bf16 rate but costs ~2% rel err where
# it owns a query's dominant key (C in e4m3); bf16 PV costs ~0.28%. The
# k-contraction is split in halves: the low half is always fp8; the high
# half is fp8 only for the first of the two batches per core. A query's
# argmax key falls in an fp8 half w.p. 1 (batch 0) / 0.5 (batch 1), so
# the global L2 rel err ~= sqrt((0.0201^2 + 0.0144^2)/2) ~= 0.0175,
# under the 2e-2 gate, while 3/4 of the PV work runs at fp8 speed.
def hi_fp8(b):
    return b == 0


def build_bass(b_loc, tq, tc, h, n_cores=N_CORES):
    """Build the per-core Bass graph. All cores run the same graph (SPMD)."""
    d = 2 * h
    ho = h
    n_qt = tq // 128       # q tiles
    n_kt = tc // 128       # k tiles
    n_hc = h // 128        # h chunks
    n_dc = d // 128        # d chunks (contraction for proj)
    kb = min(512, tc)      # QK rhs block (fp32 moving-operand max)
    n_kb = tc // kb
    hob = min(512, ho)     # proj output block
    n_hob = ho // hob
    SUPER = 2              # q-tiles per super-iteration
    assert n_qt % SUPER == 0
    n_s = n_qt // SUPER
    sq = SUPER * 128       # q columns per super-iteration
    qg = min(4, n_hc)      # f32 transposes packed per PSUM bank
    pg = min(8, n_kt)      # bf16 transposes packed per PSUM bank

    nc = bacc.Bacc("TRN2", target_bir_lowering=False, debug=False,
                   num_devices=n_cores)

    q_ext = nc.declare_dram_parameter("q", [b_loc, tq, h], F32R, isOutput=False)
    c_ext = nc.declare_dram_parameter("c", [b_loc, tc, h], F32R, isOutput=False)
    wt_ext = nc.declare_dram_parameter("wt", [d, ho], F32, isOutput=False)
    idf_ext = nc.declare_dram_parameter("idf", [128, 128], F32, isOutput=False)
    idr_ext = nc.declare_dram_parameter("idr", [128, 128], F32R, isOutput=False)
    idb_ext = nc.declare_dram_parameter("idb", [128, 128], BF16, isOutput=False)
    ones_ext = nc.declare_dram_parameter("ones", [1, 128], F32R, isOutput=False)
    out_ext = nc.declare_dram_parameter("out", [b_loc, tq, ho], F32, isOutput=True)

    with tile.TileContext(nc) as tc_:
        with (
            tc_.tile_pool(name="const", bufs=1) as const_pool,
            tc_.tile_pool(name="stage", bufs=5) as stage_pool,
            tc_.tile_pool(name="ct", bufs=1) as ct_pool,
            tc_.tile_pool(name="clo", bufs=1) as clo_pool,
            tc_.tile_pool(name="chi", bufs=1) as chi_pool,
            tc_.tile_pool(name="qt", bufs=2) as qt_pool,
            tc_.tile_pool(name="p", bufs=3) as p_pool,
            tc_.tile_pool(name="ptlo", bufs=1) as ptlo_pool,
            tc_.tile_pool(name="pthi", bufs=1) as pthi_pool,
            tc_.tile_pool(name="comb", bufs=2) as comb_pool,
            tc_.tile_pool(name="ostage", bufs=2) as out_pool,
            tc_.tile_pool(name="stats", bufs=12) as stats_pool,
            tc_.tile_pool(name="rrow", bufs=2) as rrow_pool,
            tc_.tile_pool(name="rcpb", bufs=2) as rcpb_pool,
            tc_.tile_pool(name="ps_s", bufs=1, space="PSUM") as ps_s,
            tc_.tile_pool(name="ps_tp", bufs=2, space="PSUM") as ps_tp,
            tc_.tile_pool(name="ps_mm", bufs=2, space="PSUM") as ps_mm,
        ):
            # --- constants: identities + W^T (bf16) + ones row ---
            idf = const_pool.tile([128, 128], F32, tag="idf")
            nc.sync.dma_start(idf[:], idf_ext[:])
            idr = const_pool.tile([128, 128], F32R, tag="idr")
            nc.sync.dma_start(idr[:], idr_ext[:])
            idb = const_pool.tile([128, 128], BF16, tag="idb")
            nc.sync.dma_start(idb[:], idb_ext[:])
            ones_r = const_pool.tile([1, 128], F32R, tag="ones")
            nc.sync.dma_start(ones_r[:], ones_ext[:])

            wt_bf = const_pool.tile([128, n_dc * ho], BF16, tag="wtbf")

            def emit_wt_chunk(phase):
                for dc in range(4 * phase, 4 * (phase + 1)):
                    ws = stage_pool.tile([128, ho], F32, tag="stage",
                                         name=f"ws_{dc}")
                    nc.sync.dma_start(ws[:], wt_ext[dc * 128:(dc + 1) * 128, :])
                    if dc % 2 == 0:
                        nc.vector.tensor_copy(
                            wt_bf[:, dc * ho:(dc + 1) * ho], ws[:])
                    else:
                        nc.scalar.copy(wt_bf[:, dc * ho:(dc + 1) * ho], ws[:])

            p_tiles = {}      # (b, t) -> unnormalized quantized P tile
            rcp_tiles = {}    # (b, t) -> [128, 1] reciprocal row sums
            combT_map = {}    # s -> combT tile of current batch
            pt_map = {}       # s -> P^T tile of current batch
            rcpb_map = {}     # s -> [128, sq] broadcast reciprocal tile

            def emit_qtr(b, s, ti, qs=None):
                """Q load + QT transposes; returns qt_t for the QK stage."""
                t = s * SUPER + ti
                combT = combT_map[(b, s)]
                comb_r = combT.rearrange("p (dc q) -> p dc q", q=sq)
                if qs is None:
                    qs = stage_pool.tile([128, h], F32R, tag="stage",
                                         name=f"qs_{b}_{t}")
                    nc.sync.dma_start(qs[:], q_ext[b, t * 128:(t + 1) * 128, :])
                qt_t = qt_pool.tile([128, h], F32R, tag="qt",
                                    name=f"qt_{b}_{t}")
                for g in range(n_hc // qg):
                    tq4 = ps_tp.tile([128, qg * 128], F32R, tag="tp",
                                     name=f"tq4_{b}_{t}_{g}")
                    for j in range(qg):
                        hc = qg * g + j
                        nc.tensor.transpose(
                            tq4[:, j * 128:(j + 1) * 128],
                            qs[:, hc * 128:(hc + 1) * 128], idr[:])
                    dst = qt_t[:, g * qg * 128:(g + 1) * qg * 128]
                    if g % 2 == 0:
                        nc.scalar.copy(dst, tq4[:])
                    else:
                        nc.vector.tensor_copy(dst, tq4[:])
                nc.vector.tensor_copy(
                    comb_r[:, n_hc: 2 * n_hc, ti * 128:(ti + 1) * 128],
                    qt_t.rearrange("p (j c) -> p j c", c=128)[:])
                return qt_t

            def emit_qk_block(b, t, qt_t, ct_all, kbi, s_ps):
                """One kb-wide column block of the QK matmuls (hc sweep)."""
                for hc in range(n_hc):
                    lhs = qt_t[:, hc * 128:(hc + 1) * 128]
                    rhs = ct_all[:, hc * tc + kbi * kb:
                                 hc * tc + (kbi + 1) * kb]
                    nc.tensor.matmul(
                        s_ps[:, kbi * kb:(kbi + 1) * kb], lhs, rhs,
                        start=(hc == 0), stop=(hc == n_hc - 1))

            def emit_softmax(b, t, s_ps):
                """Softmax chain on a finished scores PSUM tile.

                exp output is the UNNORMALIZED quantized P (max ~= 1);
                the row-sum (of exact exp values) is accumulated into
                l_tot and its reciprocal kept for the PV-drain
                normalization."""
                negm = stats_pool.tile([128, 1], F32, tag="negm",
                                       name=f"negm_{b}_{t}")
                nc.vector.reduce_max(
                    negm[:], s_ps[:], axis=mybir.AxisListType.X, negate=True)
                l_tot = stats_pool.tile([128, 1], F32, tag="ltot",
                                        name=f"lt_{b}_{t}")
                nc.vector.memset(l_tot[:], 0.0)
                p = p_pool.tile([128, tc], BF16, tag="p", name=f"p_{b}_{t}")
                nc.scalar.activation(
                    p[:], s_ps[:], mybir.ActivationFunctionType.Exp,
                    bias=negm[:], scale=1.0, accum_out=l_tot[:])
                rcp = stats_pool.tile([128, 1], F32, tag="rcp",
                                      name=f"rcp_{b}_{t}")
                nc.vector.reciprocal(rcp[:], l_tot[:])
                p_tiles[(b, t)] = p
                rcp_tiles[(b, t)] = rcp

            def emit_qk_softmax(b, s, ti, qt_t, ct_all):
                t = s * SUPER + ti
                s_ps = ps_s.tile([128, tc], F32, tag="s", name=f"s_{b}_{t}")
                for hc in range(n_hc):
                    for kbi in range(n_kb):
                        lhs = qt_t[:, hc * 128:(hc + 1) * 128]
                        rhs = ct_all[:, hc * tc + kbi * kb:
                                     hc * tc + (kbi + 1) * kb]
                        nc.tensor.matmul(
                            s_ps[:, kbi * kb:(kbi + 1) * kb], lhs, rhs,
                            start=(hc == 0), stop=(hc == n_hc - 1))
                emit_softmax(b, t, s_ps)

            def emit_rcpb_row(b, s):
                """Transpose the two rcp [128,1] columns into one row."""
                row_ps = ps_tp.tile([128, qg * 128], F32, tag="tp",
                                    name=f"rrow_{b}_{s}")
                for ti in range(SUPER):
                    rcp = rcp_tiles.pop((b, s * SUPER + ti))
                    nc.tensor.transpose(
                        row_ps[0:1, ti * 128:(ti + 1) * 128], rcp[:], idf[:])
                row_sb = rrow_pool.tile([1, sq], F32R, tag="rrow",
                                        name=f"rrs_{b}_{s}")
                nc.scalar.copy(row_sb[:], row_ps[0:1, 0:sq])
                return row_sb

            def emit_rcpb_bcast(b, s, row_sb):
                """Ones outer-product broadcast of 1/rowsum to [128, sq]."""
                bc_ps = ps_mm.tile([128, sq], F32, tag="mm",
                                   name=f"rbc_{b}_{s}")
                nc.tensor.matmul(bc_ps[:], ones_r[:], row_sb[:],
                                 start=True, stop=True)
                rcpb = rcpb_pool.tile([128, sq], F32, tag="rcpb",
                                      name=f"rcpb_{b}_{s}")
                nc.vector.tensor_copy(rcpb[:], bc_ps[:])
                rcpb_map[(b, s)] = rcpb

            def emit_pt(b, s):
                """P^T for super s: bf16 PE transposes packed into PSUM
                banks, drained by wide ACT/DVE copies that cast to the
                per-half PV dtype."""
                nk2 = n_kt // 2
                assert pg == nk2
                pt_lo = ptlo_pool.tile([128, nk2 * sq], FP8, tag="ptlo",
                                       name=f"ptlo_{b}_{s}")
                pt_hi = pthi_pool.tile([128, nk2 * sq],
                                       FP8 if hi_fp8(b) else BF16,
                                       tag="pthi", name=f"pthi_{b}_{s}")
                lo_r = pt_lo.rearrange("p (k q) -> p k q", q=sq)
                hi_r = pt_hi.rearrange("p (k q) -> p k q", q=sq)
                ps = [p_tiles.pop((b, s * SUPER + ti)) for ti in range(SUPER)]
                for g in range(n_kt // pg):
                    tgt_r = lo_r if g == 0 else hi_r
                    for ti in range(SUPER):
                        tp8 = ps_tp.tile([128, pg * 128], BF16, tag="tp",
                                         name=f"tp8_{b}_{s}_{ti}_{g}")
                        for j in range(pg):
                            kt = pg * g + j
                            nc.tensor.transpose(
                                tp8[:, j * 128:(j + 1) * 128],
                                ps[ti][:, kt * 128:(kt + 1) * 128], idb[:])
                        if ti % 2 == 0:
                            nc.scalar.copy(
                                tgt_r[:, 0:pg, ti * 128:(ti + 1) * 128],
                                tp8.rearrange("p (j c) -> p j c", c=128)[:])
                        else:
                            nc.vector.tensor_copy(
                                tgt_r[:, 0:pg, ti * 128:(ti + 1) * 128],
                                tp8.rearrange("p (j c) -> p j c", c=128)[:])
                pt_map[(b, s)] = (pt_lo, pt_hi)

            def emit_pv(b, s, c_lo, c_hi):
                """PV matmuls: mix^T chunks into combT for super s.

                Low k-half always fp8 DoubleRow (adjacent k-tile pairs,
                2x rate); high half DR or bf16 per batch. Drain
                multiplies by the rcpb broadcast (renormalize)."""
                nk2 = n_kt // 2
                combT = combT_map[(b, s)]
                pt_lo, pt_hi = pt_map.pop((b, s))
                rcpb = rcpb_map.pop((b, s))
                clo_r = c_lo.rearrange("p (k h2) -> p k h2", h2=h)
                chi_r = c_hi.rearrange("p (k h2) -> p k h2", h2=h)
                plo_r = pt_lo.rearrange("p (k q) -> p k q", q=sq)
                phi_r = pt_hi.rearrange("p (k q) -> p k q", q=sq)
                for hc in range(n_hc):
                    mm = ps_mm.tile([128, sq], F32, tag="mm",
                                    name=f"mm_{b}_{s}_{hc}")
                    for kt in range(0, nk2, 2):
                        nc.tensor.matmul(
                            mm[:],
                            clo_r[:, kt:kt + 2, hc * 128:(hc + 1) * 128],
                            plo_r[:, kt:kt + 2, :],
                            start=(kt == 0), stop=False,
                            perf_mode=mybir.MatmulPerfMode.DoubleRow)
                    if hi_fp8(b):
                        for kt in range(0, nk2, 2):
                            nc.tensor.matmul(
                                mm[:],
                                chi_r[:, kt:kt + 2, hc * 128:(hc + 1) * 128],
                                phi_r[:, kt:kt + 2, :],
                                start=False, stop=(kt == nk2 - 2),
                                perf_mode=mybir.MatmulPerfMode.DoubleRow)
                    else:
                        for kt in range(nk2):
                            nc.tensor.matmul(
                                mm[:],
                                c_hi[:, kt * h + hc * 128:
                                     kt * h + (hc + 1) * 128],
                                pt_hi[:, kt * sq:(kt + 1) * sq],
                                start=False, stop=(kt == nk2 - 1))
                    nc.vector.tensor_mul(
                        combT[:, hc * sq:(hc + 1) * sq], mm[:], rcpb[:])

            def emit_proj(b, s):
                """Projection + tanh + store for both tiles of super s."""
                combT = combT_map.pop((b, s))
                for ti in range(SUPER):
                    t = s * SUPER + ti
                    ostage = out_pool.tile([128, ho], F32, tag="ostage",
                                           name=f"os_{b}_{t}")
                    for hb in range(n_hob):
                        pr = ps_mm.tile([128, hob], F32, tag="mm",
                                        name=f"pr_{b}_{t}_{hb}")
                        for dc in range(n_dc):
                            nc.tensor.matmul(
                                pr[:],
                                combT[:, dc * sq + ti * 128:
                                      dc * sq + (ti + 1) * 128],
                                wt_bf[:, dc * ho + hb * hob:
                                      dc * ho + (hb + 1) * hob],
                                start=(dc == 0), stop=(dc == n_dc - 1))
                        nc.scalar.activation(
                            ostage[:, hb * hob:(hb + 1) * hob], pr[:],
                            mybir.ActivationFunctionType.Tanh)
                    nc.sync.dma_start(
                        out_ext[b, t * 128:(t + 1) * 128, :], ostage[:])

            q_pre_map = {}
            cs_pre_map = {}
            for b in range(b_loc):
                # prefetch the first super's Q tiles ahead of the C DMAs so
                # the first Qtr transposes are not stuck behind 16 MB of C/W
                q_pre = q_pre_map.pop(b, None)
                if q_pre is None:
                    q_pre = []
                    for ti in range(SUPER):
                        qp = stage_pool.tile([128, h], F32R, tag="stage",
                                             name=f"qpre_{b}_{ti}")
                        nc.sync.dma_start(qp[:],
                                          q_ext[b, ti * 128:(ti + 1) * 128, :])
                        q_pre.append(qp)
                # --- batch setup: CT (f32r, [h, k]) and C (bf16, [k, h]) ---
                ct_all = ct_pool.tile([128, n_hc * tc], F32R, tag="ct",
                                      name=f"ct_{b}")
                ct_r = ct_all.rearrange("p (hc k) -> p hc k", k=tc)
                c_lo = clo_pool.tile([128, (n_kt // 2) * h], FP8,
                                     tag="clo", name=f"clo_{b}")
                c_hi = chi_pool.tile([128, (n_kt // 2) * h],
                                     FP8 if hi_fp8(b) else BF16,
                                     tag="chi", name=f"chi_{b}")

                def emit_cs_dma(bb, kt, split=False):
                    cs = stage_pool.tile([128, h], F32R, tag="stage",
                                         name=f"cs_{bb}_{kt}")
                    if split:
                        nc.sync.dma_start(
                            cs[:, 0:h // 2],
                            c_ext[bb, kt * 128:(kt + 1) * 128, 0:h // 2])
                        nc.sync.dma_start(
                            cs[:, h // 2:h],
                            c_ext[bb, kt * 128:(kt + 1) * 128, h // 2:h])
                    else:
                        nc.sync.dma_start(
                            cs[:], c_ext[bb, kt * 128:(kt + 1) * 128, :])
                    return cs

                def emit_c_setup(kt):
                    cs = cs_pre_map.pop((b, kt), None)
                    if cs is None:
                        cs = emit_cs_dma(b, kt, split=(b == 0 and kt < 2))
                    nk2 = n_kt // 2
                    if kt < nk2:
                        dst = c_lo[:, kt * h:(kt + 1) * h]
                    else:
                        dst = c_hi[:, (kt - nk2) * h:(kt - nk2 + 1) * h]
                    if kt % 2 == 0:
                        nc.vector.tensor_copy(dst, cs[:])
                    else:
                        nc.scalar.copy(dst, cs[:])
                    for g in range(n_hc // qg):
                        tc4 = ps_tp.tile([128, qg * 128], F32R, tag="tp",
                                         name=f"tc4_{b}_{kt}_{g}")
                        for j in range(qg):
                            hc = qg * g + j
                            nc.tensor.transpose(
                                tc4[:, j * 128:(j + 1) * 128],
                                cs[:, hc * 128:(hc + 1) * 128], idr[:])
                        dst = ct_r[:, qg * g: qg * (g + 1),
                                   kt * 128:(kt + 1) * 128]
                        src = tc4.rearrange("p (j c) -> p j c", c=128)[:]
                        if (g + kt) % 2 == 1:
                            nc.scalar.copy(dst, src)
                        else:
                            nc.vector.tensor_copy(dst, src)

                # first half of C, then the first Q-transpose (fills the
                # DMA-paced window), then the rest of C
                for kt in range(n_kt // 2):
                    emit_c_setup(kt)
                combT_map[(b, 0)] = comb_pool.tile(
                    [128, n_dc * sq], BF16, tag="comb", name=f"cb_{b}_0")
                qt0_first = emit_qtr(b, 0, 0, qs=q_pre[0])
                for kt in range(n_kt // 2, n_kt):
                    emit_c_setup(kt)
                if b == 0:
                    for ph in range(4):
                        emit_wt_chunk(ph)

                # --- pipelined main loop ---
                for s in range(n_s):
                    if s > 0:
                        row_sb = emit_rcpb_row(b, s - 1)
                        combT_map[(b, s)] = comb_pool.tile(
                            [128, n_dc * sq], BF16, tag="comb",
                            name=f"cb_{b}_{s}")
                        qt0 = emit_qtr(b, s, 0)
                        emit_rcpb_bcast(b, s - 1, row_sb)
                        emit_pt(b, s - 1)
                    else:
                        qt0 = qt0_first
                    emit_qk_softmax(b, s, 0, qt0, ct_all)
                    qt1 = emit_qtr(b, s, 1, qs=q_pre[1] if s == 0 else None)
                    if s > 0:
                        emit_pv(b, s - 1, c_lo, c_hi)
                    emit_qk_softmax(b, s, 1, qt1, ct_all)
                    if s > 0:
                        emit_proj(b, s - 1)
                    if b + 1 < b_loc and s >= n_s - 2:
                        for kt in range(2 * (s - (n_s - 2)),
                                        2 * (s - (n_s - 2)) + 2):
                            cs_pre_map[(b + 1, kt)] = emit_cs_dma(b + 1, kt)
                row_sb = emit_rcpb_row(b, n_s - 1)
                emit_rcpb_bcast(b, n_s - 1, row_sb)
                emit_pt(b, n_s - 1)
                # prefetch the next batch's first C tiles + Q during the tail
                if b + 1 < b_loc:
                    qp2 = []
                    for ti in range(SUPER):
                        qp = stage_pool.tile([128, h], F32R, tag="stage",
                                             name=f"qpre_{b + 1}_{ti}")
                        nc.sync.dma_start(
                            qp[:], q_ext[b + 1, ti * 128:(ti + 1) * 128, :])
                        qp2.append(qp)
                    q_pre_map[b + 1] = qp2
                    for kt in range(4, 6):
                        cs_pre_map[(b + 1, kt)] = emit_cs_dma(b + 1, kt)
                emit_pv(b, n_s - 1, c_lo, c_hi)
                if b + 1 < b_loc:
                    for kt in range(6, 8):
                        cs_pre_map[(b + 1, kt)] = emit_cs_dma(b + 1, kt)
                emit_proj(b, n_s - 1)

    nc.compile()
    return nc


_NC_CACHE = {}


def _get_nc(b_loc, tq, tc, h):
    key = (b_loc, tq, tc, h)
    if key not in _NC_CACHE:
        _NC_CACHE[key] = build_bass(b_loc, tq, tc, h)
    return _NC_CACHE[key]


def make_in_maps(query, context, W_attn, n_cores=N_CORES):
    b = query.shape[0]
    b_loc = b // n_cores
    wt = np.ascontiguousarray(W_attn.T.astype(np.float32))
    idf = np.eye(128, dtype=np.float32)
    idb = np.eye(128).astype(ml_dtypes.bfloat16)
    in_maps = []
    for i in range(n_cores):
        in_maps.append({
            "q": np.ascontiguousarray(
                query[i * b_loc:(i + 1) * b_loc].astype(np.float32)),
            "c": np.ascontiguousarray(
                context[i * b_loc:(i + 1) * b_loc].astype(np.float32)),
            "wt": wt,
            "idf": idf,
            "idr": idf,
            "idb": idb,
            "ones": np.ones((1, 128), dtype=np.float32),
        })
    return in_maps


def kernel(query, context, W_attn, _trace=False, _trace_kwargs=None):
    b, tq, h = query.shape
    tc = context.shape[1]
    b_loc = b // N_CORES
    nc = _get_nc(b_loc, tq, tc, h)
    in_maps = make_in_maps(query, context, W_attn)
    res = run_bass_kernel_spmd(
        nc, in_maps, core_ids=list(range(N_CORES)), trace=_trace,
        **(_trace_kwargs or {}))
    out = np.concatenate([res.results[i]["out"] for i in range(N_CORES)], axis=0)
    if _trace:
        return out, res
    return out



# revision 3
# speedup vs baseline: 1.0981x; 1.0981x over previous
"""Trainium2 8-core kernel for batched attention + concat projection.

Reference computation (per batch b):
    scores = Q @ C^T                  [TQ, TC]
    A      = softmax(scores, axis=-1)
    mix    = A @ C                    [TQ, H]
    out    = tanh(concat([mix, Q]) @ W^T)   [TQ, H]

Distribution: pure data-parallel over batch (B=16 across 8 cores, 2
batches per core), W replicated. No collectives needed.

v2 design (vs the f32r baseline):
  - ALL layout work is done on the host: Q^T / C^T ship pre-transposed
    (fp16), C ships pre-quantized (fp8/bf16) in [k, h] tile layout, and
    W^T ships pre-cast fp16.  The device runs zero f32 transposes and
    zero dtype-staging copies (the baseline burned ~135 us of PE time
    on f32 transposes and ~130 us of DVE on staging casts).
  - QK runs in fp16 (numerically indistinguishable from f32 here:
    sim rel-err identical to 4 decimals).  No fp32 matmuls anywhere
    means fp16/bf16 LDWEIGHTS keep FWL and hide behind the matmul
    stream, where the baseline's fp32 QK was LDWEIGHTS-paced
    (289 ns/MM for a 216 ns ideal).
  - SUPER=4 (512 q columns per super-iteration) so the fp8 DoubleRow
    PV matmuls run at free-dim 512, where DR's ~2x rate is not
    LDW-limited (baseline FD=256 measured 157 ns per 2-k-tile pair;
    FD=512 target ~224 ns per pair covering 2x the columns).
  - softmax over free axis k: DVE reduce_max(negate) -> ACT exp with
    per-partition bias, bf16 output (unnormalized, max ~= 1) and f32
    row-sum accumulator -> DVE reciprocal.  Normalization is folded
    into the PV PSUM drain via a [128, sq] broadcast of 1/rowsum
    (PE: transpose rcp columns to a row + ones outer-product).
  - P^T via bf16 PE transposes packed 8-per-PSUM-bank; drains cast to
    the per-half PV dtype (fp8 lo always; hi fp8 on batch 0, bf16 on
    batch 1 -- same error budget as the baseline, measured 1.74e-2
    against the 2e-2 gate).
  - proj reads the concat's Q half straight out of the fp16 Q^T tile
    (no comb assembly for that half) and runs fp16 x fp16 -> tanh.
  - Stages of super s-1 are interleaved between the QK blocks of
    super s so the in-order PE stream always has ready work while the
    softmax chain (DVE reduce_max -> ACT exp) of the current q-tile
    completes; scores PSUM is WAR-recycled per q-tile.
"""

import numpy as np
import ml_dtypes

import concourse.bacc as bacc
import concourse.tile as tile
import concourse.mybir as mybir
from concourse.bass_utils import run_bass_kernel_spmd

F32 = mybir.dt.float32
F32R = mybir.dt.float32r
F16 = mybir.dt.float16
BF16 = mybir.dt.bfloat16
FP8 = mybir.dt.float8e4

N_CORES = 8
B, TQ, TC, H = 16, 2048, 2048, 1024

SUPER = 4              # q-tiles per super-iteration
PG = 8                 # bf16 transposes packed per PSUM bank


# fp8 PV costs ~2% rel err where it owns a query's dominant key; bf16
# costs ~0.3%.  lo k-half always fp8; hi half fp8 only on local batch 0.
# Measured composite: 1.74e-2 (gate 2e-2).
def hi_fp8(b):
    return b == 0


def build_bass(b_loc, tq, tc, h, n_cores=N_CORES):
    """Build the per-core Bass graph. All cores run the same graph (SPMD)."""
    d = 2 * h
    ho = h
    n_qt = tq // 128       # q tiles
    n_kt = tc // 128       # k tiles
    n_hc = h // 128        # h chunks
    n_dc = d // 128        # d chunks (contraction for proj)
    kb = 512               # QK rhs block
    n_kb = tc // kb
    hob = 512              # proj output block
    n_hob = ho // hob
    assert n_qt % SUPER == 0
    n_s = n_qt // SUPER
    sq = SUPER * 128       # q columns per super-iteration
    nk2 = n_kt // 2
    assert PG == nk2

    nc = bacc.Bacc("TRN2", target_bir_lowering=False, debug=False,
                   num_devices=n_cores)

    # host-prepped inputs (see make_in_maps for layouts)
    qt_ext = nc.declare_dram_parameter("qt", [b_loc, n_s, 128, n_hc * sq],
                                       F16, isOutput=False)
    ct_ext = nc.declare_dram_parameter("ct", [b_loc, n_kb, 128, n_hc * kb],
                                       F16, isOutput=False)
    clo_ext = nc.declare_dram_parameter("clo", [b_loc, 128, nk2 * h], FP8,
                                        isOutput=False)
    chi8_ext = nc.declare_dram_parameter("chi8", [128, nk2 * h], FP8,
                                         isOutput=False)
    chib_ext = nc.declare_dram_parameter("chib", [128, nk2 * h], BF16,
                                         isOutput=False)
    wt_ext = nc.declare_dram_parameter("wt", [128, n_dc * ho], F16,
                                       isOutput=False)
    idf_ext = nc.declare_dram_parameter("idf", [128, 128], F32, isOutput=False)
    idb_ext = nc.declare_dram_parameter("idb", [128, 128], BF16, isOutput=False)
    ones_ext = nc.declare_dram_parameter("ones", [1, 128], F32R, isOutput=False)
    out_ext = nc.declare_dram_parameter("out", [b_loc, tq, ho], F32,
                                        isOutput=True)

    with tile.TileContext(nc) as tc_:
        with (
            tc_.tile_pool(name="const", bufs=1) as const_pool,
            tc_.tile_pool(name="ct", bufs=1) as ct_pool,
            tc_.tile_pool(name="qt", bufs=3) as qt_pool,
            tc_.tile_pool(name="clo", bufs=1) as clo_pool,
            tc_.tile_pool(name="chi8", bufs=1) as chi8_pool,
            tc_.tile_pool(name="chib", bufs=1) as chib_pool,
            tc_.tile_pool(name="p", bufs=6) as p_pool,
            tc_.tile_pool(name="ptlo", bufs=1) as ptlo_pool,
            tc_.tile_pool(name="pthi8", bufs=1) as pthi8_pool,
            tc_.tile_pool(name="pthib", bufs=1) as pthib_pool,
            tc_.tile_pool(name="comb", bufs=2) as comb_pool,
            tc_.tile_pool(name="ostage", bufs=3) as out_pool,
            tc_.tile_pool(name="stats", bufs=18) as stats_pool,
            tc_.tile_pool(name="rrow", bufs=2) as rrow_pool,
            tc_.tile_pool(name="rcpb", bufs=2) as rcpb_pool,
            tc_.tile_pool(name="ps_s", bufs=1, space="PSUM") as ps_s,
            tc_.tile_pool(name="ps_tp", bufs=2, space="PSUM") as ps_tp,
            tc_.tile_pool(name="ps_mm", bufs=2, space="PSUM") as ps_mm,
        ):
            # --- constants ---
            idf = const_pool.tile([128, 128], F32, tag="idf")
            nc.sync.dma_start(idf[:], idf_ext[:])
            idb = const_pool.tile([128, 128], BF16, tag="idb")
            nc.sync.dma_start(idb[:], idb_ext[:])
            ones_r = const_pool.tile([1, 128], F32R, tag="ones")
            nc.sync.dma_start(ones_r[:], ones_ext[:])
            wt_t = const_pool.tile([128, n_dc * ho], F16, tag="wt")
            for half in range(2):
                nc.sync.dma_start(
                    wt_t[:, half * 8 * ho:(half + 1) * 8 * ho],
                    wt_ext[:, half * 8 * ho:(half + 1) * 8 * ho])

            p_tiles = {}      # (b, t) -> unnormalized bf16 P tile
            rcp_tiles = {}    # (b, t) -> [128, 1] reciprocal row sums
            qt_map = {}       # (b, s) -> fp16 Q^T tile (QK lhs + proj Q-half)
            combT_map = {}    # (b, s) -> fp16 mix^T tile
            pt_map = {}       # (b, s) -> (pt_lo, pt_hi)
            rcpb_map = {}     # (b, s) -> [128, sq] broadcast reciprocal

            def emit_qt_dma(b, s):
                qt_t = qt_pool.tile([128, n_hc * sq], F16, tag="qt",
                                    name=f"qt_{b}_{s}")
                nc.sync.dma_start(qt_t[:], qt_ext[b, s])
                qt_map[(b, s)] = qt_t

            def emit_ct_dma(b):
                ct_t = ct_pool.tile([128, n_kb * n_hc * kb], F16, tag="ct",
                                    name=f"ct_{b}")
                for kbi in range(n_kb):
                    nc.sync.dma_start(
                        ct_t[:, kbi * n_hc * kb:(kbi + 1) * n_hc * kb],
                        ct_ext[b, kbi])
                return ct_t

            def emit_c_dma(b):
                clo_t = clo_pool.tile([128, nk2 * h], FP8, tag="clo",
                                      name=f"clo_{b}")
                nc.sync.dma_start(clo_t[:], clo_ext[b])
                if hi_fp8(b):
                    chi_t = chi8_pool.tile([128, nk2 * h], FP8, tag="chi8",
                                           name=f"chi_{b}")
                    nc.sync.dma_start(chi_t[:], chi8_ext[:])
                else:
                    chi_t = chib_pool.tile([128, nk2 * h], BF16, tag="chib",
                                           name=f"chi_{b}")
                    nc.sync.dma_start(chi_t[:], chib_ext[:])
                return clo_t, chi_t

            def emit_qk_softmax(b, s, ti, ct_t):
                """Scores for q-tile (s, ti) + softmax chain.

                exp output is the UNNORMALIZED bf16 P (max ~= 1); the
                row-sum of exact exp values accumulates into l_tot and
                its reciprocal is kept for the PV-drain renormalize."""
                t = s * SUPER + ti
                qt_t = qt_map[(b, s)]
                s_ps = ps_s.tile([128, tc], F32, tag="s", name=f"s_{b}_{t}")
                for hc in range(n_hc):
                    lhs = qt_t[:, hc * sq + ti * 128:hc * sq + (ti + 1) * 128]
                    for kbi in range(n_kb):
                        rhs = ct_t[:, kbi * n_hc * kb + hc * kb:
                                   kbi * n_hc * kb + (hc + 1) * kb]
                        nc.tensor.matmul(
                            s_ps[:, kbi * kb:(kbi + 1) * kb], lhs, rhs,
                            start=(hc == 0), stop=(hc == n_hc - 1))
                negm = stats_pool.tile([128, 1], F32, tag="negm",
                                       name=f"negm_{b}_{t}")
                nc.vector.reduce_max(
                    negm[:], s_ps[:], axis=mybir.AxisListType.X, negate=True)
                l_tot = stats_pool.tile([128, 1], F32, tag="ltot",
                                        name=f"lt_{b}_{t}")
                nc.vector.memset(l_tot[:], 0.0)
                p = p_pool.tile([128, tc], BF16, tag="p", name=f"p_{b}_{t}")
                nc.scalar.activation(
                    p[:], s_ps[:], mybir.ActivationFunctionType.Exp,
                    bias=negm[:], scale=1.0, accum_out=l_tot[:])
                rcp = stats_pool.tile([128, 1], F32, tag="rcp",
                                      name=f"rcp_{b}_{t}")
                nc.vector.reciprocal(rcp[:], l_tot[:])
                p_tiles[(b, t)] = p
                rcp_tiles[(b, t)] = rcp

            def emit_rcpb_row(b, s):
                """Transpose the SUPER rcp [128,1] columns into one row."""
                row_ps = ps_tp.tile([128, sq], F32, tag="tp",
                                    name=f"rrow_{b}_{s}")
                for ti in range(SUPER):
                    rcp = rcp_tiles.pop((b, s * SUPER + ti))
                    nc.tensor.transpose(
                        row_ps[0:1, ti * 128:(ti + 1) * 128], rcp[:], idf[:])
                row_sb = rrow_pool.tile([1, sq], F32R, tag="rrow",
                                        name=f"rrs_{b}_{s}")
                nc.scalar.copy(row_sb[:], row_ps[0:1, 0:sq])
                return row_sb

            def emit_rcpb_bcast(b, s, row_sb):
                """Ones outer-product broadcast of 1/rowsum to [128, sq]."""
                bc_ps = ps_mm.tile([128, sq], F32, tag="mm",
                                   name=f"rbc_{b}_{s}")
                nc.tensor.matmul(bc_ps[:], ones_r[:], row_sb[:],
                                 start=True, stop=True)
                rcpb = rcpb_pool.tile([128, sq], F32, tag="rcpb",
                                      name=f"rcpb_{b}_{s}")
                nc.vector.tensor_copy(rcpb[:], bc_ps[:])
                rcpb_map[(b, s)] = rcpb

            def emit_pt(b, s):
                """P^T for super s: bf16 PE transposes packed into PSUM
                banks, drained by wide ACT/DVE copies casting each
                k-half to its PV dtype."""
                pt_lo = ptlo_pool.tile([128, nk2 * sq], FP8, tag="ptlo",
                                       name=f"ptlo_{b}_{s}")
                if hi_fp8(b):
                    pt_hi = pthi8_pool.tile([128, nk2 * sq], FP8,
                                            tag="pthi8", name=f"pthi_{b}_{s}")
                else:
                    pt_hi = pthib_pool.tile([128, nk2 * sq], BF16,
                                            tag="pthib", name=f"pthi_{b}_{s}")
                lo_r = pt_lo.rearrange("p (k q) -> p k q", q=sq)
                hi_r = pt_hi.rearrange("p (k q) -> p k q", q=sq)
                ps = [p_tiles.pop((b, s * SUPER + ti)) for ti in range(SUPER)]
                for g in range(n_kt // PG):
                    tgt_r = lo_r if g == 0 else hi_r
                    for ti in range(SUPER):
                        tp8 = ps_tp.tile([128, PG * 128], BF16, tag="tp",
                                         name=f"tp8_{b}_{s}_{ti}_{g}")
                        for j in range(PG):
                            kt = PG * g + j
                            nc.tensor.transpose(
                                tp8[:, j * 128:(j + 1) * 128],
                                ps[ti][:, kt * 128:(kt + 1) * 128], idb[:])
                        if ti % 2 == 0:
                            nc.scalar.copy(
                                tgt_r[:, 0:PG, ti * 128:(ti + 1) * 128],
                                tp8.rearrange("p (j c) -> p j c", c=128)[:])
                        else:
                            nc.vector.tensor_copy(
                                tgt_r[:, 0:PG, ti * 128:(ti + 1) * 128],
                                tp8.rearrange("p (j c) -> p j c", c=128)[:])
                pt_map[(b, s)] = (pt_lo, pt_hi)

            def emit_pv(b, s, c_lo, c_hi, hcs):
                """PV matmuls for h-chunks `hcs`: mix^T into combT.

                Low k-half always fp8 DoubleRow (adjacent k-tile pairs);
                high half DR or bf16 per batch.  Drain multiplies by the
                rcpb broadcast (renormalize)."""
                combT = combT_map[(b, s)]
                pt_lo, pt_hi = pt_map[(b, s)]
                rcpb = rcpb_map[(b, s)]
                clo_r = c_lo.rearrange("p (k h2) -> p k h2", h2=h)
                chi_r = c_hi.rearrange("p (k h2) -> p k h2", h2=h)
                plo_r = pt_lo.rearrange("p (k q) -> p k q", q=sq)
                phi_r = pt_hi.rearrange("p (k q) -> p k q", q=sq)
                for hc in hcs:
                    mm = ps_mm.tile([128, sq], F32, tag="mm",
                                    name=f"mm_{b}_{s}_{hc}")
                    for kt in range(0, nk2, 2):
                        nc.tensor.matmul(
                            mm[:],
                            clo_r[:, kt:kt + 2, hc * 128:(hc + 1) * 128],
                            plo_r[:, kt:kt + 2, :],
                            start=(kt == 0), stop=False,
                            perf_mode=mybir.MatmulPerfMode.DoubleRow)
                    if hi_fp8(b):
                        for kt in range(0, nk2, 2):
                            nc.tensor.matmul(
                                mm[:],
                                chi_r[:, kt:kt + 2, hc * 128:(hc + 1) * 128],
                                phi_r[:, kt:kt + 2, :],
                                start=False, stop=(kt == nk2 - 2),
                                perf_mode=mybir.MatmulPerfMode.DoubleRow)
                    else:
                        for kt in range(nk2):
                            nc.tensor.matmul(
                                mm[:],
                                c_hi[:, kt * h + hc * 128:
                                     kt * h + (hc + 1) * 128],
                                pt_hi[:, kt * sq:(kt + 1) * sq],
                                start=False, stop=(kt == nk2 - 1))
                    nc.vector.tensor_mul(
                        combT[:, hc * sq:(hc + 1) * sq], mm[:], rcpb[:])

            def emit_proj(b, s, tis):
                """Projection + tanh + store for q-tiles `tis` of super s.

                The concat's Q half is read straight from the fp16 Q^T
                tile; the mix half from combT."""
                combT = combT_map[(b, s)]
                qt_t = qt_map[(b, s)]
                for ti in tis:
                    t = s * SUPER + ti
                    ostage = out_pool.tile([128, ho], F32, tag="ostage",
                                           name=f"os_{b}_{t}")
                    for hb in range(n_hob):
                        pr = ps_mm.tile([128, hob], F32, tag="mm",
                                        name=f"pr_{b}_{t}_{hb}")
                        for dc in range(n_dc):
                            if dc < n_hc:
                                lhs = combT[:, dc * sq + ti * 128:
                                            dc * sq + (ti + 1) * 128]
                            else:
                                lhs = qt_t[:, (dc - n_hc) * sq + ti * 128:
                                           (dc - n_hc) * sq + (ti + 1) * 128]
                            nc.tensor.matmul(
                                pr[:], lhs,
                                wt_t[:, dc * ho + hb * hob:
                                     dc * ho + (hb + 1) * hob],
                                start=(dc == 0), stop=(dc == n_dc - 1))
                        nc.scalar.activation(
                            ostage[:, hb * hob:(hb + 1) * hob], pr[:],
                            mybir.ActivationFunctionType.Tanh)
                    nc.sync.dma_start(
                        out_ext[b, t * 128:(t + 1) * 128, :], ostage[:])

            # ------------------------------------------------------------
            # pipelined main program
            # ------------------------------------------------------------
            prefetched = {}   # b -> (ct_t, clo_t, chi_t); s=0 QKs pre-emitted
            for b in range(b_loc):
                if b in prefetched:
                    ct_t, clo_t, chi_t = prefetched.pop(b)
                    s0_done = True
                else:
                    ct_t = emit_ct_dma(b)
                    emit_qt_dma(b, 0)
                    clo_t, chi_t = emit_c_dma(b)
                    s0_done = False

                for s in range(n_s):
                    if s + 1 < n_s:
                        emit_qt_dma(b, s + 1)
                    if s == 0 and s0_done:
                        continue
                    if s > 0:
                        row_sb = emit_rcpb_row(b, s - 1)
                        emit_rcpb_bcast(b, s - 1, row_sb)
                        combT_map[(b, s - 1)] = comb_pool.tile(
                            [128, n_hc * sq], F16, tag="comb",
                            name=f"cb_{b}_{s - 1}")
                    emit_qk_softmax(b, s, 0, ct_t)
                    if s > 0:
                        emit_pt(b, s - 1)
                        emit_pv(b, s - 1, clo_t, chi_t, range(0, 2))
                    emit_qk_softmax(b, s, 1, ct_t)
                    if s > 0:
                        emit_pv(b, s - 1, clo_t, chi_t, range(2, n_hc))
                    emit_qk_softmax(b, s, 2, ct_t)
                    if s > 0:
                        emit_proj(b, s - 1, [0, 1])
                    emit_qk_softmax(b, s, 3, ct_t)
                    if s > 0:
                        emit_proj(b, s - 1, [2, 3])
                        pt_map.pop((b, s - 1))
                        rcpb_map.pop((b, s - 1))
                        combT_map.pop((b, s - 1))

                # --- batch tail: last super's tail stages, interleaved
                # with the next batch's prefetch DMAs and (for b+1) its
                # first QK blocks ---
                sl = n_s - 1
                row_sb = emit_rcpb_row(b, sl)
                emit_rcpb_bcast(b, sl, row_sb)
                combT_map[(b, sl)] = comb_pool.tile(
                    [128, n_hc * sq], F16, tag="comb", name=f"cb_{b}_{sl}")
                emit_pt(b, sl)
                nb = b + 1
                if nb < b_loc:
                    # WAR-safe: every reader of ct_t/qt(b,*) is emitted
                    ct_next = emit_ct_dma(nb)
                    emit_qt_dma(nb, 0)
                emit_pv(b, sl, clo_t, chi_t, range(n_hc))
                if nb < b_loc:
                    clo_n, chi_n = emit_c_dma(nb)
                    prefetched[nb] = (ct_next, clo_n, chi_n)
                    emit_qk_softmax(nb, 0, 0, ct_next)
                    emit_proj(b, sl, [0])
                    emit_qk_softmax(nb, 0, 1, ct_next)
                    emit_proj(b, sl, [1])
                    emit_qk_softmax(nb, 0, 2, ct_next)
                    emit_proj(b, sl, [2, 3])
                    emit_qk_softmax(nb, 0, 3, ct_next)
                else:
                    emit_proj(b, sl, [0, 1, 2, 3])
                pt_map.pop((b, sl))
                rcpb_map.pop((b, sl))
                combT_map.pop((b, sl))

            # mark the prefetched first-super QKs of the last batch as
            # consumed bookkeeping (handled inside the loop above via
            # p_tiles/rcp_tiles maps)

    nc.compile()
    return nc


_NC_CACHE = {}


def _get_nc(b_loc, tq, tc, h):
    key = (b_loc, tq, tc, h)
    if key not in _NC_CACHE:
        _NC_CACHE[key] = build_bass(b_loc, tq, tc, h)
    return _NC_CACHE[key]


def make_in_maps(query, context, W_attn, n_cores=N_CORES):
    b = query.shape[0]
    b_loc = b // n_cores
    tq, h = query.shape[1], query.shape[2]
    tc = context.shape[1]
    n_s = tq // (SUPER * 128)
    sq = SUPER * 128
    n_hc = h // 128
    n_kb = tc // 512
    nk2 = (tc // 128) // 2
    n_dc = 2 * h // 128
    F8NP = ml_dtypes.float8_e4m3
    BFNP = ml_dtypes.bfloat16

    q = np.ascontiguousarray(query).reshape(n_cores, b_loc, tq, h)
    c = np.ascontiguousarray(context).reshape(n_cores, b_loc, tc, h)

    # qt[i, b, s, p, hc*sq + j] = Q[i, b, s*sq + j, hc*128 + p]
    qt = np.ascontiguousarray(
        q.reshape(n_cores, b_loc, n_s, sq, n_hc, 128)
        .transpose(0, 1, 2, 5, 4, 3)
        .reshape(n_cores, b_loc, n_s, 128, n_hc * sq)
        .astype(np.float16))
    # ct[i, b, kbi, p, hc*512 + j] = C[i, b, kbi*512 + j, hc*128 + p]
    ct = np.ascontiguousarray(
        c.reshape(n_cores, b_loc, n_kb, 512, n_hc, 128)
        .transpose(0, 1, 2, 5, 4, 3)
        .reshape(n_cores, b_loc, n_kb, 128, n_hc * 512)
        .astype(np.float16))
    # c in [k-tile, h] layout: cl[i, b, p, kt*h + j] = C[i, b, kt*128+p, j]
    ckh = (c.reshape(n_cores, b_loc, 2 * nk2, 128, h)
           .transpose(0, 1, 3, 2, 4))  # [i, b, 128, 2*nk2, h]
    clo = np.ascontiguousarray(
        ckh[:, :, :, :nk2].reshape(n_cores, b_loc, 128, nk2 * h)
        .astype(F8NP))
    chi = ckh[:, :, :, nk2:].reshape(n_cores, b_loc, 128, nk2 * h)
    chi8 = np.ascontiguousarray(chi[:, 0].astype(F8NP))
    chib = np.ascontiguousarray(chi[:, 1].astype(BFNP))
    # wt[p, dc*h + j] = W_attn[j, dc*128 + p]
    wt = np.ascontiguousarray(
        np.ascontiguousarray(W_attn.T)
        .reshape(n_dc, 128, h).transpose(1, 0, 2)
        .reshape(128, n_dc * h).astype(np.float16))

    idf = np.eye(128, dtype=np.float32)
    idb = np.eye(128).astype(BFNP)
    ones = np.ones((1, 128), dtype=np.float32)

    in_maps = []
    for i in range(n_cores):
        in_maps.append({
            "qt": qt[i],
            "ct": ct[i],
            "clo": clo[i],
            "chi8": chi8[i],
            "chib": chib[i],
            "wt": wt,
            "idf": idf,
            "idb": idb,
            "ones": ones,
        })
    return in_maps


def kernel(query, context, W_attn, _trace=False, _trace_kwargs=None):
    b, tq, h = query.shape
    tc = context.shape[1]
    b_loc = b // N_CORES
    nc = _get_nc(b_loc, tq, tc, h)
    in_maps = make_in_maps(query, context, W_attn)
    res = run_bass_kernel_spmd(
        nc, in_maps, core_ids=list(range(N_CORES)), trace=_trace,
        **(_trace_kwargs or {}))
    out = np.concatenate([res.results[i]["out"] for i in range(N_CORES)],
                         axis=0)
    if _trace:
        return out, res
    return out


# revision 13
# speedup vs baseline: 1.4229x; 1.2958x over previous
"""Trainium2 8-core kernel for batched attention + concat projection.

Reference computation (per batch b):
    scores = Q @ C^T                  [TQ, TC]
    A      = softmax(scores, axis=-1)
    mix    = A @ C                    [TQ, H]
    out    = tanh(concat([mix, Q]) @ W^T)   [TQ, H]

Distribution: pure data-parallel over batch (B=16 across 8 cores, 2
batches per core), W replicated. No collectives needed.

v2 design (vs the f32r baseline):
  - ALL layout work is done on the host: Q^T / C^T ship pre-transposed
    (fp16), C ships pre-quantized (fp8/bf16) in [k, h] tile layout, and
    W^T ships pre-cast fp16.  The device runs zero f32 transposes and
    zero dtype-staging copies (the baseline burned ~135 us of PE time
    on f32 transposes and ~130 us of DVE on staging casts).
  - QK runs in fp16 (numerically indistinguishable from f32 here:
    sim rel-err identical to 4 decimals).  No fp32 matmuls anywhere
    means fp16/bf16 LDWEIGHTS keep FWL and hide behind the matmul
    stream, where the baseline's fp32 QK was LDWEIGHTS-paced
    (289 ns/MM for a 216 ns ideal).
  - SUPER=4 (512 q columns per super-iteration) so the fp8 DoubleRow
    PV matmuls run at free-dim 512, where DR's ~2x rate is not
    LDW-limited (baseline FD=256 measured 157 ns per 2-k-tile pair;
    FD=512 target ~224 ns per pair covering 2x the columns).
  - softmax over free axis k: DVE reduce_max(negate) -> ACT exp with
    per-partition bias, bf16 output (unnormalized, max ~= 1) and f32
    row-sum accumulator -> DVE reciprocal.  Normalization is folded
    into the PV PSUM drain via a [128, sq] broadcast of 1/rowsum
    (PE: transpose rcp columns to a row + ones outer-product).
  - P^T via bf16 PE transposes packed 8-per-PSUM-bank; drains cast to
    the per-half PV dtype (fp8 lo always; hi fp8 on batch 0, bf16 on
    batch 1 -- same error budget as the baseline, measured 1.74e-2
    against the 2e-2 gate).
  - proj reads the concat's Q half straight out of the fp16 Q^T tile
    (no comb assembly for that half) and runs fp16 x fp16 -> tanh.
  - Stages of super s-1 are interleaved between the QK blocks of
    super s so the in-order PE stream always has ready work while the
    softmax chain (DVE reduce_max -> ACT exp) of the current q-tile
    completes; scores PSUM is WAR-recycled per q-tile.
"""

import numpy as np
import ml_dtypes

import concourse.bacc as bacc
import concourse.tile as tile
import concourse.mybir as mybir
from concourse.bass_utils import run_bass_kernel_spmd

F32 = mybir.dt.float32
F32R = mybir.dt.float32r
F16 = mybir.dt.float16
BF16 = mybir.dt.bfloat16
FP8 = mybir.dt.float8e4

N_CORES = 8
B, TQ, TC, H = 16, 2048, 2048, 1024

SUPER = 4              # q-tiles per super-iteration
PG = 8                 # bf16 transposes packed per PSUM bank


# fp8 PV costs ~2% rel err where it owns a query's dominant key; bf16
# costs ~0.3%.  lo k-half always fp8; hi half fp8 only on local batch 0.
# Measured composite: 1.74e-2 (gate 2e-2).
def hi_fp8(b):
    return b == 0


def build_bass(b_loc, tq, tc, h, n_cores=N_CORES):
    """Build the per-core Bass graph. All cores run the same graph (SPMD)."""
    d = 2 * h
    ho = h
    n_qt = tq // 128       # q tiles
    n_kt = tc // 128       # k tiles
    n_hc = h // 128        # h chunks
    n_dc = d // 128        # d chunks (contraction for proj)
    kb = 512               # QK rhs block
    n_kb = tc // kb
    hob = 512              # proj output block
    n_hob = ho // hob
    assert n_qt % SUPER == 0
    n_s = n_qt // SUPER
    sq = SUPER * 128       # q columns per super-iteration
    nk2 = n_kt // 2
    assert PG == nk2

    nc = bacc.Bacc("TRN2", target_bir_lowering=False, debug=False,
                   num_devices=n_cores)

    # host-prepped inputs (see make_in_maps for layouts)
    qt_ext = nc.declare_dram_parameter("qt", [b_loc, n_s, 128, n_hc * sq],
                                       F16, isOutput=False)
    ct_ext = nc.declare_dram_parameter("ct", [b_loc, n_kb, 128, n_hc * kb],
                                       F16, isOutput=False)
    clo_ext = nc.declare_dram_parameter("clo", [b_loc, 128, nk2 * h], FP8,
                                        isOutput=False)
    chi8_ext = nc.declare_dram_parameter("chi8", [128, nk2 * h], FP8,
                                         isOutput=False)
    chib_ext = nc.declare_dram_parameter("chib", [128, nk2 * h], BF16,
                                         isOutput=False)
    wt_ext = nc.declare_dram_parameter("wt", [128, n_dc * ho], F16,
                                       isOutput=False)
    idf_ext = nc.declare_dram_parameter("idf", [128, 128], F32, isOutput=False)
    idb_ext = nc.declare_dram_parameter("idb", [128, 128], BF16, isOutput=False)
    ones_ext = nc.declare_dram_parameter("ones", [1, 128], F32R, isOutput=False)
    out_ext = nc.declare_dram_parameter("out", [b_loc, tq, ho], F32,
                                        isOutput=True)

    with tile.TileContext(nc) as tc_:
        with (
            tc_.tile_pool(name="const", bufs=1) as const_pool,
            tc_.tile_pool(name="ct", bufs=1) as ct_pool,
            tc_.tile_pool(name="qt", bufs=3) as qt_pool,
            tc_.tile_pool(name="clo", bufs=1) as clo_pool,
            tc_.tile_pool(name="chi8", bufs=1) as chi8_pool,
            tc_.tile_pool(name="chib", bufs=1) as chib_pool,
            tc_.tile_pool(name="p", bufs=6) as p_pool,
            tc_.tile_pool(name="ptlo", bufs=1) as ptlo_pool,
            tc_.tile_pool(name="pthi8", bufs=1) as pthi8_pool,
            tc_.tile_pool(name="pthib", bufs=1) as pthib_pool,
            tc_.tile_pool(name="comb", bufs=2) as comb_pool,
            tc_.tile_pool(name="ostage", bufs=3) as out_pool,
            tc_.tile_pool(name="stats", bufs=24) as stats_pool,
            tc_.tile_pool(name="rrow", bufs=2) as rrow_pool,
            tc_.tile_pool(name="rcpb", bufs=2) as rcpb_pool,
            tc_.tile_pool(name="ps_s", bufs=4, space="PSUM") as ps_s,
            tc_.tile_pool(name="ps_tp", bufs=2, space="PSUM") as ps_tp,
            tc_.tile_pool(name="ps_mm", bufs=2, space="PSUM") as ps_mm,
        ):
            # --- constants (wt DMA deferred off the startup critical path) ---
            idf = const_pool.tile([128, 128], F32, tag="idf")
            idb = const_pool.tile([128, 128], BF16, tag="idb")
            ones_r = const_pool.tile([1, 128], F32R, tag="ones")
            wt_t = const_pool.tile([128, n_dc * ho], F16, tag="wt")

            def emit_const_dma():
                nc.sync.dma_start(idf[:], idf_ext[:])
                nc.sync.dma_start(idb[:], idb_ext[:])
                nc.sync.dma_start(ones_r[:], ones_ext[:])

            def emit_wt_dma():
                for half in range(2):
                    nc.sync.dma_start(
                        wt_t[:, half * 8 * ho:(half + 1) * 8 * ho],
                        wt_ext[:, half * 8 * ho:(half + 1) * 8 * ho])

            p_tiles = {}      # (b, t) -> unnormalized bf16 P tile
            rcp_tiles = {}    # (b, t) -> [128, 1] reciprocal row sums
            qt_map = {}       # (b, s) -> fp16 Q^T tile (QK lhs + proj Q-half)
            combT_map = {}    # (b, s) -> fp16 mix^T tile
            pt_map = {}       # (b, s) -> (pt_lo, pt_hi)
            rcpb_map = {}     # (b, s) -> [128, sq] broadcast reciprocal

            def emit_qt_dma(b, s):
                qt_t = qt_pool.tile([128, n_hc * sq], F16, tag="qt",
                                    name=f"qt_{b}_{s}")
                nc.sync.dma_start(qt_t[:], qt_ext[b, s])
                qt_map[(b, s)] = qt_t

            def emit_ct_dma(b):
                ct_t = ct_pool.tile([128, n_kb * n_hc * kb], F16, tag="ct",
                                    name=f"ct_{b}")
                for kbi in range(n_kb):
                    nc.sync.dma_start(
                        ct_t[:, kbi * n_hc * kb:(kbi + 1) * n_hc * kb],
                        ct_ext[b, kbi])
                return ct_t

            def emit_c_dma(b):
                clo_t = clo_pool.tile([128, nk2 * h], FP8, tag="clo",
                                      name=f"clo_{b}")
                nc.sync.dma_start(clo_t[:], clo_ext[b])
                if hi_fp8(b):
                    chi_t = chi8_pool.tile([128, nk2 * h], FP8, tag="chi8",
                                           name=f"chi_{b}")
                    nc.sync.dma_start(chi_t[:], chi8_ext[:])
                else:
                    chi_t = chib_pool.tile([128, nk2 * h], BF16, tag="chib",
                                           name=f"chi_{b}")
                    nc.sync.dma_start(chi_t[:], chib_ext[:])
                return clo_t, chi_t

            def emit_qk_softmax(b, s, ti, ct_t):
                """Scores for q-tile (s, ti) + softmax chain.

                kbi-outer: each 512-col PSUM bank finishes its hc
                accumulation early, so its partial row-max runs on DVE
                while the next bank's matmuls stream, and each exp
                releases its bank for the next q-tile's QK (per-bank
                tiles from a bufs=4 pool) with ~1.5 us latency instead
                of a 4.4 us whole-tile WAR.

                exp output is the UNNORMALIZED bf16 P (max ~= 1); the
                row-sums of exact exp values accumulate into l_tot and
                the reciprocal is kept for the PV-drain renormalize."""
                t = s * SUPER + ti
                qt_t = qt_map[(b, s)]
                pm = stats_pool.tile([128, n_kb], F32, tag="pm",
                                     name=f"pm_{b}_{t}")
                banks = []
                for kbi in range(n_kb):
                    sb = ps_s.tile([128, kb], F32, tag="s",
                                   name=f"s_{b}_{t}_{kbi}")
                    for hc in range(n_hc):
                        lhs = qt_t[:, hc * sq + ti * 128:
                                   hc * sq + (ti + 1) * 128]
                        rhs = ct_t[:, kbi * n_hc * kb + hc * kb:
                                   kbi * n_hc * kb + (hc + 1) * kb]
                        nc.tensor.matmul(
                            sb[:], lhs, rhs,
                            start=(hc == 0), stop=(hc == n_hc - 1))
                    nc.vector.reduce_max(
                        pm[:, kbi:kbi + 1], sb[:], axis=mybir.AxisListType.X)
                    banks.append(sb)
                negm = stats_pool.tile([128, 1], F32, tag="negm",
                                       name=f"negm_{b}_{t}")
                nc.vector.reduce_max(
                    negm[:], pm[:], axis=mybir.AxisListType.X, negate=True)
                lacc = stats_pool.tile([128, n_kb], F32, tag="lacc",
                                       name=f"lacc_{b}_{t}")
                nc.vector.memset(lacc[:], 0.0)
                p = p_pool.tile([128, tc], BF16, tag="p", name=f"p_{b}_{t}")
                for kbi, sb in enumerate(banks):
                    nc.scalar.activation(
                        p[:, kbi * kb:(kbi + 1) * kb], sb[:],
                        mybir.ActivationFunctionType.Exp,
                        bias=negm[:], scale=1.0,
                        accum_out=lacc[:, kbi:kbi + 1])
                l_tot = stats_pool.tile([128, 1], F32, tag="ltot",
                                        name=f"lt_{b}_{t}")
                nc.vector.reduce_sum(l_tot[:], lacc[:],
                                     axis=mybir.AxisListType.X)
                rcp = stats_pool.tile([128, 1], F32, tag="rcp",
                                      name=f"rcp_{b}_{t}")
                nc.vector.reciprocal(rcp[:], l_tot[:])
                p_tiles[(b, t)] = p
                rcp_tiles[(b, t)] = rcp

            def emit_rcpb_row(b, s):
                """Transpose the SUPER rcp [128,1] columns into one row."""
                row_ps = ps_tp.tile([128, sq], F32, tag="tp",
                                    name=f"rrow_{b}_{s}")
                for ti in range(SUPER):
                    rcp = rcp_tiles.pop((b, s * SUPER + ti))
                    nc.tensor.transpose(
                        row_ps[0:1, ti * 128:(ti + 1) * 128], rcp[:], idf[:])
                row_sb = rrow_pool.tile([1, sq], F32R, tag="rrow",
                                        name=f"rrs_{b}_{s}")
                nc.scalar.copy(row_sb[:], row_ps[0:1, 0:sq])
                return row_sb

            def emit_rcpb_bcast(b, s, row_sb):
                """Ones outer-product broadcast of 1/rowsum to [128, sq]."""
                bc_ps = ps_mm.tile([128, sq], F32, tag="mm",
                                   name=f"rbc_{b}_{s}")
                nc.tensor.matmul(bc_ps[:], ones_r[:], row_sb[:],
                                 start=True, stop=True)
                rcpb = rcpb_pool.tile([128, sq], F32, tag="rcpb",
                                      name=f"rcpb_{b}_{s}")
                nc.vector.tensor_copy(rcpb[:], bc_ps[:])
                rcpb_map[(b, s)] = rcpb

            def emit_pt(b, s):
                """P^T for super s: bf16 PE transposes packed into PSUM
                banks, drained by wide ACT/DVE copies casting each
                k-half to its PV dtype."""
                pt_lo = ptlo_pool.tile([128, nk2 * sq], FP8, tag="ptlo",
                                       name=f"ptlo_{b}_{s}")
                if hi_fp8(b):
                    pt_hi = pthi8_pool.tile([128, nk2 * sq], FP8,
                                            tag="pthi8", name=f"pthi_{b}_{s}")
                else:
                    pt_hi = pthib_pool.tile([128, nk2 * sq], BF16,
                                            tag="pthib", name=f"pthi_{b}_{s}")
                ps = [p_tiles.pop((b, s * SUPER + ti)) for ti in range(SUPER)]
                # pack one k-tile PAIR x all SUPER q-tiles per PSUM bank so
                # the drain is a single fully-CONTIGUOUS [128, 2*sq] copy
                # into the [k, q] pt layout (strided drains measured 2x
                # slower and made P^T drain-paced).
                for half in range(2):
                    tgt = pt_lo if half == 0 else pt_hi
                    for kp in range(nk2 // 2):
                        tp8 = ps_tp.tile([128, 2 * sq], BF16, tag="tp",
                                         name=f"tp8_{b}_{s}_{half}_{kp}")
                        for j in range(2):
                            kt = half * nk2 + kp * 2 + j
                            for ti in range(SUPER):
                                nc.tensor.transpose(
                                    tp8[:, (j * SUPER + ti) * 128:
                                        (j * SUPER + ti + 1) * 128],
                                    ps[ti][:, kt * 128:(kt + 1) * 128],
                                    idb[:])
                        dst = tgt[:, kp * 2 * sq:(kp + 1) * 2 * sq]
                        if kp % 2 == 0:
                            nc.scalar.copy(dst, tp8[:])
                        else:
                            nc.vector.tensor_copy(dst, tp8[:])
                pt_map[(b, s)] = (pt_lo, pt_hi)

            def emit_pv(b, s, c_lo, c_hi, hcs):
                """PV matmuls for h-chunks `hcs`: mix^T into combT.

                Low k-half always fp8 DoubleRow (adjacent k-tile pairs);
                high half DR or bf16 per batch.  Drain multiplies by the
                rcpb broadcast (renormalize)."""
                combT = combT_map[(b, s)]
                pt_lo, pt_hi = pt_map[(b, s)]
                rcpb = rcpb_map[(b, s)]
                clo_r = c_lo.rearrange("p (k h2) -> p k h2", h2=h)
                chi_r = c_hi.rearrange("p (k h2) -> p k h2", h2=h)
                plo_r = pt_lo.rearrange("p (k q) -> p k q", q=sq)
                phi_r = pt_hi.rearrange("p (k q) -> p k q", q=sq)
                for hc in hcs:
                    mm = ps_mm.tile([128, sq], F32, tag="mm",
                                    name=f"mm_{b}_{s}_{hc}")
                    for kt in range(0, nk2, 2):
                        nc.tensor.matmul(
                            mm[:],
                            clo_r[:, kt:kt + 2, hc * 128:(hc + 1) * 128],
                            plo_r[:, kt:kt + 2, :],
                            start=(kt == 0), stop=False,
                            perf_mode=mybir.MatmulPerfMode.DoubleRow)
                    if hi_fp8(b):
                        for kt in range(0, nk2, 2):
                            nc.tensor.matmul(
                                mm[:],
                                chi_r[:, kt:kt + 2, hc * 128:(hc + 1) * 128],
                                phi_r[:, kt:kt + 2, :],
                                start=False, stop=(kt == nk2 - 2),
                                perf_mode=mybir.MatmulPerfMode.DoubleRow)
                    else:
                        for kt in range(nk2):
                            nc.tensor.matmul(
                                mm[:],
                                c_hi[:, kt * h + hc * 128:
                                     kt * h + (hc + 1) * 128],
                                pt_hi[:, kt * sq:(kt + 1) * sq],
                                start=False, stop=(kt == nk2 - 1))
                    nc.vector.tensor_mul(
                        combT[:, hc * sq:(hc + 1) * sq], mm[:], rcpb[:])

            def emit_proj(b, s, tis):
                """Projection + tanh + store for q-tiles `tis` of super s.

                The concat's Q half is read straight from the fp16 Q^T
                tile; the mix half from combT."""
                combT = combT_map[(b, s)]
                qt_t = qt_map[(b, s)]
                for ti in tis:
                    t = s * SUPER + ti
                    ostage = out_pool.tile([128, ho], F32, tag="ostage",
                                           name=f"os_{b}_{t}")
                    for hb in range(n_hob):
                        pr = ps_mm.tile([128, hob], F32, tag="mm",
                                        name=f"pr_{b}_{t}_{hb}")
                        for dc in range(n_dc):
                            if dc < n_hc:
                                lhs = combT[:, dc * sq + ti * 128:
                                            dc * sq + (ti + 1) * 128]
                            else:
                                lhs = qt_t[:, (dc - n_hc) * sq + ti * 128:
                                           (dc - n_hc) * sq + (ti + 1) * 128]
                            nc.tensor.matmul(
                                pr[:], lhs,
                                wt_t[:, dc * ho + hb * hob:
                                     dc * ho + (hb + 1) * hob],
                                start=(dc == 0), stop=(dc == n_dc - 1))
                        nc.scalar.activation(
                            ostage[:, hb * hob:(hb + 1) * hob], pr[:],
                            mybir.ActivationFunctionType.Tanh)
                    nc.sync.dma_start(
                        out_ext[b, t * 128:(t + 1) * 128, :], ostage[:])

            # ------------------------------------------------------------
            # pipelined main program
            # ------------------------------------------------------------
            prefetched = {}   # b -> (ct_t, clo_t, chi_t); s=0 QKs pre-emitted
            for b in range(b_loc):
                if b in prefetched:
                    ct_t, clo_t, chi_t = prefetched.pop(b)
                    s0_done = True
                else:
                    # startup critical path: only ct + qt(0) gate the
                    # first QK.  clo/chi/wt (6 MB) are deferred into the
                    # first super's emission (not needed until s=1).
                    ct_t = emit_ct_dma(b)
                    emit_qt_dma(b, 0)
                    emit_const_dma()
                    clo_t = chi_t = None
                    s0_done = False

                for s in range(n_s):
                    if s + 1 < n_s:
                        emit_qt_dma(b, s + 1)
                    if s == 0 and s0_done:
                        continue
                    if s > 0:
                        row_sb = emit_rcpb_row(b, s - 1)
                        emit_rcpb_bcast(b, s - 1, row_sb)
                        combT_map[(b, s - 1)] = comb_pool.tile(
                            [128, n_hc * sq], F16, tag="comb",
                            name=f"cb_{b}_{s - 1}")
                    emit_qk_softmax(b, s, 0, ct_t)
                    if s > 0:
                        emit_pt(b, s - 1)
                        emit_pv(b, s - 1, clo_t, chi_t, range(0, 2))
                    emit_qk_softmax(b, s, 1, ct_t)
                    if s == 0 and clo_t is None:
                        clo_t, chi_t = emit_c_dma(b)
                    if s > 0:
                        emit_pv(b, s - 1, clo_t, chi_t, range(2, n_hc))
                    emit_qk_softmax(b, s, 2, ct_t)
                    if s > 0:
                        emit_proj(b, s - 1, [0, 1])
                    emit_qk_softmax(b, s, 3, ct_t)
                    if s == 0:
                        emit_wt_dma()
                    if s > 0:
                        emit_proj(b, s - 1, [2, 3])
                        pt_map.pop((b, s - 1))
                        rcpb_map.pop((b, s - 1))
                        combT_map.pop((b, s - 1))

                # --- batch tail: last super's tail stages, interleaved
                # with the next batch's prefetch DMAs and (for b+1) its
                # first QK blocks ---
                sl = n_s - 1
                row_sb = emit_rcpb_row(b, sl)
                emit_rcpb_bcast(b, sl, row_sb)
                combT_map[(b, sl)] = comb_pool.tile(
                    [128, n_hc * sq], F16, tag="comb", name=f"cb_{b}_{sl}")
                emit_pt(b, sl)
                nb = b + 1
                if nb < b_loc:
                    # WAR-safe: every reader of ct_t/qt(b,*) is emitted
                    ct_next = emit_ct_dma(nb)
                    emit_qt_dma(nb, 0)
                emit_pv(b, sl, clo_t, chi_t, range(n_hc))
                if nb < b_loc:
                    clo_n, chi_n = emit_c_dma(nb)
                    prefetched[nb] = (ct_next, clo_n, chi_n)
                    emit_qk_softmax(nb, 0, 0, ct_next)
                    emit_proj(b, sl, [0])
                    emit_qk_softmax(nb, 0, 1, ct_next)
                    emit_proj(b, sl, [1])
                    emit_qk_softmax(nb, 0, 2, ct_next)
                    emit_proj(b, sl, [2, 3])
                    emit_qk_softmax(nb, 0, 3, ct_next)
                else:
                    emit_proj(b, sl, [0, 1, 2, 3])
                pt_map.pop((b, sl))
                rcpb_map.pop((b, sl))
                combT_map.pop((b, sl))

            # mark the prefetched first-super QKs of the last batch as
            # consumed bookkeeping (handled inside the loop above via
            # p_tiles/rcp_tiles maps)

    nc.compile()
    return nc


_NC_CACHE = {}


def _get_nc(b_loc, tq, tc, h):
    key = (b_loc, tq, tc, h)
    if key not in _NC_CACHE:
        _NC_CACHE[key] = build_bass(b_loc, tq, tc, h)
    return _NC_CACHE[key]


def make_in_maps(query, context, W_attn, n_cores=N_CORES):
    b = query.shape[0]
    b_loc = b // n_cores
    tq, h = query.shape[1], query.shape[2]
    tc = context.shape[1]
    n_s = tq // (SUPER * 128)
    sq = SUPER * 128
    n_hc = h // 128
    n_kb = tc // 512
    nk2 = (tc // 128) // 2
    n_dc = 2 * h // 128
    F8NP = ml_dtypes.float8_e4m3
    BFNP = ml_dtypes.bfloat16

    q = np.ascontiguousarray(query).reshape(n_cores, b_loc, tq, h)
    c = np.ascontiguousarray(context).reshape(n_cores, b_loc, tc, h)

    # qt[i, b, s, p, hc*sq + j] = Q[i, b, s*sq + j, hc*128 + p]
    qt = np.ascontiguousarray(
        q.reshape(n_cores, b_loc, n_s, sq, n_hc, 128)
        .transpose(0, 1, 2, 5, 4, 3)
        .reshape(n_cores, b_loc, n_s, 128, n_hc * sq)
        .astype(np.float16))
    # ct[i, b, kbi, p, hc*512 + j] = C[i, b, kbi*512 + j, hc*128 + p]
    ct = np.ascontiguousarray(
        c.reshape(n_cores, b_loc, n_kb, 512, n_hc, 128)
        .transpose(0, 1, 2, 5, 4, 3)
        .reshape(n_cores, b_loc, n_kb, 128, n_hc * 512)
        .astype(np.float16))
    # c in [k-tile, h] layout: cl[i, b, p, kt*h + j] = C[i, b, kt*128+p, j]
    ckh = (c.reshape(n_cores, b_loc, 2 * nk2, 128, h)
           .transpose(0, 1, 3, 2, 4))  # [i, b, 128, 2*nk2, h]
    clo = np.ascontiguousarray(
        ckh[:, :, :, :nk2].reshape(n_cores, b_loc, 128, nk2 * h)
        .astype(F8NP))
    chi = ckh[:, :, :, nk2:].reshape(n_cores, b_loc, 128, nk2 * h)
    chi8 = np.ascontiguousarray(chi[:, 0].astype(F8NP))
    chib = np.ascontiguousarray(chi[:, 1].astype(BFNP))
    # wt[p, dc*h + j] = W_attn[j, dc*128 + p]
    wt = np.ascontiguousarray(
        np.ascontiguousarray(W_attn.T)
        .reshape(n_dc, 128, h).transpose(1, 0, 2)
        .reshape(128, n_dc * h).astype(np.float16))

    idf = np.eye(128, dtype=np.float32)
    idb = np.eye(128).astype(BFNP)
    ones = np.ones((1, 128), dtype=np.float32)

    in_maps = []
    for i in range(n_cores):
        in_maps.append({
            "qt": qt[i],
            "ct": ct[i],
            "clo": clo[i],
            "chi8": chi8[i],
            "chib": chib[i],
            "wt": wt,
            "idf": idf,
            "idb": idb,
            "ones": ones,
        })
    return in_maps


def kernel(query, context, W_attn, _trace=False, _trace_kwargs=None):
    b, tq, h = query.shape
    tc = context.shape[1]
    b_loc = b // N_CORES
    nc = _get_nc(b_loc, tq, tc, h)
    in_maps = make_in_maps(query, context, W_attn)
    res = run_bass_kernel_spmd(
        nc, in_maps, core_ids=list(range(N_CORES)), trace=_trace,
        **(_trace_kwargs or {}))
    out = np.concatenate([res.results[i]["out"] for i in range(N_CORES)],
                         axis=0)
    if _trace:
        return out, res
    return out


# revision 21
# speedup vs baseline: 1.4596x; 1.0258x over previous
"""Trainium2 8-core kernel for batched attention + concat projection.

Reference computation (per batch b):
    scores = Q @ C^T                  [TQ, TC]
    A      = softmax(scores, axis=-1)
    mix    = A @ C                    [TQ, H]
    out    = tanh(concat([mix, Q]) @ W^T)   [TQ, H]

Distribution: pure data-parallel over batch (B=16 across 8 cores, 2
batches per core), W replicated. No collectives needed.

v2 design (vs the f32r baseline):
  - ALL layout work is done on the host: Q^T / C^T ship pre-transposed
    (fp16), C ships pre-quantized (fp8/bf16) in [k, h] tile layout, and
    W^T ships pre-cast fp16.  The device runs zero f32 transposes and
    zero dtype-staging copies (the baseline burned ~135 us of PE time
    on f32 transposes and ~130 us of DVE on staging casts).
  - QK runs in fp16 (numerically indistinguishable from f32 here:
    sim rel-err identical to 4 decimals).  No fp32 matmuls anywhere
    means fp16/bf16 LDWEIGHTS keep FWL and hide behind the matmul
    stream, where the baseline's fp32 QK was LDWEIGHTS-paced
    (289 ns/MM for a 216 ns ideal).
  - SUPER=4 (512 q columns per super-iteration) so the fp8 DoubleRow
    PV matmuls run at free-dim 512, where DR's ~2x rate is not
    LDW-limited (baseline FD=256 measured 157 ns per 2-k-tile pair;
    FD=512 target ~224 ns per pair covering 2x the columns).
  - softmax over free axis k: DVE reduce_max(negate) -> ACT exp with
    per-partition bias, bf16 output (unnormalized, max ~= 1) and f32
    row-sum accumulator -> DVE reciprocal.  Normalization is folded
    into the PV PSUM drain via a [128, sq] broadcast of 1/rowsum
    (PE: transpose rcp columns to a row + ones outer-product).
  - P^T via bf16 PE transposes packed 8-per-PSUM-bank; drains cast to
    the per-half PV dtype (fp8 lo always; hi fp8 on batch 0, bf16 on
    batch 1 -- same error budget as the baseline, measured 1.74e-2
    against the 2e-2 gate).
  - proj reads the concat's Q half straight out of the fp16 Q^T tile
    (no comb assembly for that half) and runs fp16 x fp16 -> tanh.
  - Stages of super s-1 are interleaved between the QK blocks of
    super s so the in-order PE stream always has ready work while the
    softmax chain (DVE reduce_max -> ACT exp) of the current q-tile
    completes; scores PSUM is WAR-recycled per q-tile.
"""

import numpy as np
import ml_dtypes

import concourse.bacc as bacc
import concourse.tile as tile
import concourse.mybir as mybir
from concourse.bass_utils import run_bass_kernel_spmd

F32 = mybir.dt.float32
F32R = mybir.dt.float32r
F16 = mybir.dt.float16
BF16 = mybir.dt.bfloat16
FP8 = mybir.dt.float8e4

N_CORES = 8
B, TQ, TC, H = 16, 2048, 2048, 1024

SUPER = 4              # q-tiles per super-iteration
PG = 8                 # bf16 transposes packed per PSUM bank


# PV error is dominated by the fp8 quantization of C (P-fp8 alone costs
# only ~0.3%): sim err ~= 0.0176 * sqrt(fp8_fraction), HW ~= 1.14x sim.
# Keep the last N_KT_BF16 k-tiles of every batch in bf16 (f = 7/8):
# predicted HW rel err ~0.0187 against the 2e-2 gate.
N_KT_BF16 = 2


def build_bass(b_loc, tq, tc, h, n_cores=N_CORES):
    """Build the per-core Bass graph. All cores run the same graph (SPMD)."""
    d = 2 * h
    ho = h
    n_qt = tq // 128       # q tiles
    n_kt = tc // 128       # k tiles
    n_hc = h // 128        # h chunks
    n_dc = d // 128        # d chunks (contraction for proj)
    kb = 512               # QK rhs block
    n_kb = tc // kb
    hob = 512              # proj output block
    n_hob = ho // hob
    assert n_qt % SUPER == 0
    n_s = n_qt // SUPER
    sq = SUPER * 128       # q columns per super-iteration
    nk2 = n_kt // 2
    nkb16 = N_KT_BF16      # trailing k-tiles of the hi half kept bf16
    nk8 = nk2 - nkb16      # fp8 k-tiles in the hi half
    assert nk8 % 2 == 0 and nkb16 % 2 == 0

    nc = bacc.Bacc("TRN2", target_bir_lowering=False, debug=False,
                   num_devices=n_cores)

    # host-prepped inputs (see make_in_maps for layouts)
    qt_ext = nc.declare_dram_parameter("qt", [b_loc, n_s, 128, n_hc * sq],
                                       F16, isOutput=False)
    ct_ext = nc.declare_dram_parameter("ct", [b_loc, n_kb, 128, n_hc * kb],
                                       F16, isOutput=False)
    clo_ext = nc.declare_dram_parameter("clo", [b_loc, 128, nk2 * h], FP8,
                                        isOutput=False)
    chi8_ext = nc.declare_dram_parameter("chi8", [b_loc, 128, nk8 * h], FP8,
                                         isOutput=False)
    chib_ext = nc.declare_dram_parameter("chib", [b_loc, 128, nkb16 * h],
                                         BF16, isOutput=False)
    wt_ext = nc.declare_dram_parameter("wt", [128, n_dc * ho], F16,
                                       isOutput=False)
    idf_ext = nc.declare_dram_parameter("idf", [128, 128], F32, isOutput=False)
    idb_ext = nc.declare_dram_parameter("idb", [128, 128], BF16, isOutput=False)
    ones_ext = nc.declare_dram_parameter("ones", [1, 128], F32R, isOutput=False)
    out_ext = nc.declare_dram_parameter("out", [b_loc, tq, ho], F32,
                                        isOutput=True)

    with tile.TileContext(nc) as tc_:
        with (
            tc_.tile_pool(name="const", bufs=1) as const_pool,
            tc_.tile_pool(name="ct", bufs=1) as ct_pool,
            tc_.tile_pool(name="qt", bufs=3) as qt_pool,
            tc_.tile_pool(name="clo", bufs=1) as clo_pool,
            tc_.tile_pool(name="chi8", bufs=1) as chi8_pool,
            tc_.tile_pool(name="chib", bufs=1) as chib_pool,
            tc_.tile_pool(name="p", bufs=6) as p_pool,
            tc_.tile_pool(name="ptlo", bufs=1) as ptlo_pool,
            tc_.tile_pool(name="pthi8", bufs=1) as pthi8_pool,
            tc_.tile_pool(name="pthib", bufs=1) as pthib_pool,
            tc_.tile_pool(name="comb", bufs=2) as comb_pool,
            tc_.tile_pool(name="ostage", bufs=3) as out_pool,
            tc_.tile_pool(name="stats", bufs=24) as stats_pool,
            tc_.tile_pool(name="rrow", bufs=2) as rrow_pool,
            tc_.tile_pool(name="rcpb", bufs=2) as rcpb_pool,
            tc_.tile_pool(name="ps_s", bufs=4, space="PSUM") as ps_s,
            tc_.tile_pool(name="ps_tp", bufs=2, space="PSUM") as ps_tp,
            tc_.tile_pool(name="ps_mm", bufs=2, space="PSUM") as ps_mm,
        ):
            # --- constants (wt DMA deferred off the startup critical path) ---
            idf = const_pool.tile([128, 128], F32, tag="idf")
            idb = const_pool.tile([128, 128], BF16, tag="idb")
            ones_r = const_pool.tile([1, 128], F32R, tag="ones")
            wt_t = const_pool.tile([128, n_dc * ho], F16, tag="wt")

            def emit_const_dma():
                nc.sync.dma_start(idf[:], idf_ext[:])
                nc.sync.dma_start(idb[:], idb_ext[:])
                nc.sync.dma_start(ones_r[:], ones_ext[:])

            def emit_wt_dma():
                for half in range(2):
                    nc.sync.dma_start(
                        wt_t[:, half * 8 * ho:(half + 1) * 8 * ho],
                        wt_ext[:, half * 8 * ho:(half + 1) * 8 * ho])

            p_tiles = {}      # (b, t) -> unnormalized bf16 P tile
            rcp_tiles = {}    # (b, t) -> [128, 1] reciprocal row sums
            qt_map = {}       # (b, s) -> fp16 Q^T tile (QK lhs + proj Q-half)
            combT_map = {}    # (b, s) -> fp16 mix^T tile
            pt_map = {}       # (b, s) -> (pt_lo, pt_hi)
            rcpb_map = {}     # (b, s) -> [128, sq] broadcast reciprocal

            def emit_qt_dma(b, s):
                qt_t = qt_pool.tile([128, n_hc * sq], F16, tag="qt",
                                    name=f"qt_{b}_{s}")
                nc.sync.dma_start(qt_t[:], qt_ext[b, s])
                qt_map[(b, s)] = qt_t

            def emit_ct_dma(b):
                ct_t = ct_pool.tile([128, n_kb * n_hc * kb], F16, tag="ct",
                                    name=f"ct_{b}")
                for kbi in range(n_kb):
                    nc.sync.dma_start(
                        ct_t[:, kbi * n_hc * kb:(kbi + 1) * n_hc * kb],
                        ct_ext[b, kbi])
                return ct_t

            def emit_c_dma(b):
                clo_t = clo_pool.tile([128, nk2 * h], FP8, tag="clo",
                                      name=f"clo_{b}")
                nc.sync.dma_start(clo_t[:], clo_ext[b])
                chi_t = chi8_pool.tile([128, nk8 * h], FP8, tag="chi8",
                                       name=f"chi8_{b}")
                nc.sync.dma_start(chi_t[:], chi8_ext[b])
                chb_t = chib_pool.tile([128, nkb16 * h], BF16, tag="chib",
                                       name=f"chib_{b}")
                nc.sync.dma_start(chb_t[:], chib_ext[b])
                return clo_t, (chi_t, chb_t)

            def emit_qk_softmax(b, s, ti, ct_t):
                """Scores for q-tile (s, ti) + softmax chain.

                kbi-outer: each 512-col PSUM bank finishes its hc
                accumulation early, so its partial row-max runs on DVE
                while the next bank's matmuls stream, and each exp
                releases its bank for the next q-tile's QK (per-bank
                tiles from a bufs=4 pool) with ~1.5 us latency instead
                of a 4.4 us whole-tile WAR.

                exp output is the UNNORMALIZED bf16 P (max ~= 1); the
                row-sums of exact exp values accumulate into l_tot and
                the reciprocal is kept for the PV-drain renormalize."""
                t = s * SUPER + ti
                qt_t = qt_map[(b, s)]
                pm = stats_pool.tile([128, n_kb], F32, tag="pm",
                                     name=f"pm_{b}_{t}")
                banks = []
                for kbi in range(n_kb):
                    sb = ps_s.tile([128, kb], F32, tag="s",
                                   name=f"s_{b}_{t}_{kbi}")
                    for hc in range(n_hc):
                        lhs = qt_t[:, hc * sq + ti * 128:
                                   hc * sq + (ti + 1) * 128]
                        rhs = ct_t[:, kbi * n_hc * kb + hc * kb:
                                   kbi * n_hc * kb + (hc + 1) * kb]
                        nc.tensor.matmul(
                            sb[:], lhs, rhs,
                            start=(hc == 0), stop=(hc == n_hc - 1))
                    nc.vector.reduce_max(
                        pm[:, kbi:kbi + 1], sb[:], axis=mybir.AxisListType.X)
                    banks.append(sb)
                negm = stats_pool.tile([128, 1], F32, tag="negm",
                                       name=f"negm_{b}_{t}")
                nc.vector.reduce_max(
                    negm[:], pm[:], axis=mybir.AxisListType.X, negate=True)
                lacc = stats_pool.tile([128, n_kb], F32, tag="lacc",
                                       name=f"lacc_{b}_{t}")
                nc.vector.memset(lacc[:], 0.0)
                p = p_pool.tile([128, tc], BF16, tag="p", name=f"p_{b}_{t}")
                for kbi, sb in enumerate(banks):
                    nc.scalar.activation(
                        p[:, kbi * kb:(kbi + 1) * kb], sb[:],
                        mybir.ActivationFunctionType.Exp,
                        bias=negm[:], scale=1.0,
                        accum_out=lacc[:, kbi:kbi + 1])
                l_tot = stats_pool.tile([128, 1], F32, tag="ltot",
                                        name=f"lt_{b}_{t}")
                nc.vector.reduce_sum(l_tot[:], lacc[:],
                                     axis=mybir.AxisListType.X)
                rcp = stats_pool.tile([128, 1], F32, tag="rcp",
                                      name=f"rcp_{b}_{t}")
                nc.vector.reciprocal(rcp[:], l_tot[:])
                p_tiles[(b, t)] = p
                rcp_tiles[(b, t)] = rcp

            def emit_rcpb_row(b, s):
                """Transpose the SUPER rcp [128,1] columns into one row."""
                row_ps = ps_tp.tile([128, sq], F32, tag="tp",
                                    name=f"rrow_{b}_{s}")
                for ti in range(SUPER):
                    rcp = rcp_tiles.pop((b, s * SUPER + ti))
                    nc.tensor.transpose(
                        row_ps[0:1, ti * 128:(ti + 1) * 128], rcp[:], idf[:])
                row_sb = rrow_pool.tile([1, sq], F32R, tag="rrow",
                                        name=f"rrs_{b}_{s}")
                nc.scalar.copy(row_sb[:], row_ps[0:1, 0:sq])
                return row_sb

            def emit_rcpb_bcast(b, s, row_sb):
                """Ones outer-product broadcast of 1/rowsum to [128, sq]."""
                bc_ps = ps_mm.tile([128, sq], F32, tag="mm",
                                   name=f"rbc_{b}_{s}")
                nc.tensor.matmul(bc_ps[:], ones_r[:], row_sb[:],
                                 start=True, stop=True)
                rcpb = rcpb_pool.tile([128, sq], F32, tag="rcpb",
                                      name=f"rcpb_{b}_{s}")
                nc.vector.tensor_copy(rcpb[:], bc_ps[:])
                rcpb_map[(b, s)] = rcpb

            def emit_pt(b, s):
                """P^T for super s: bf16 PE transposes packed into PSUM
                banks, drained by wide ACT/DVE copies casting each
                k-half to its PV dtype."""
                pt_lo = ptlo_pool.tile([128, nk2 * sq], FP8, tag="ptlo",
                                       name=f"ptlo_{b}_{s}")
                pt_hi = pthi8_pool.tile([128, nk8 * sq], FP8,
                                        tag="pthi8", name=f"pthi_{b}_{s}")
                pt_hb = pthib_pool.tile([128, nkb16 * sq], BF16,
                                        tag="pthib", name=f"pthb_{b}_{s}")
                ps = [p_tiles.pop((b, s * SUPER + ti)) for ti in range(SUPER)]
                # pack one k-tile PAIR x all SUPER q-tiles per PSUM bank so
                # the drain is a single fully-CONTIGUOUS [128, 2*sq] copy
                # into the [k, q] pt layout (strided drains measured 2x
                # slower and made P^T drain-paced).
                for kp in range(n_kt // 2):
                    if kp < nk2 // 2:
                        tgt, kbase = pt_lo, 0
                    elif kp < (nk2 + nk8) // 2:
                        tgt, kbase = pt_hi, nk2
                    else:
                        tgt, kbase = pt_hb, nk2 + nk8
                    tp8 = ps_tp.tile([128, 2 * sq], BF16, tag="tp",
                                     name=f"tp8_{b}_{s}_{kp}")
                    for j in range(2):
                        kt = kp * 2 + j
                        for ti in range(SUPER):
                            nc.tensor.transpose(
                                tp8[:, (j * SUPER + ti) * 128:
                                    (j * SUPER + ti + 1) * 128],
                                ps[ti][:, kt * 128:(kt + 1) * 128],
                                idb[:])
                    dst = tgt[:, (kp * 2 - kbase) * sq:
                              (kp * 2 - kbase + 2) * sq]
                    if kp % 2 == 0:
                        nc.scalar.copy(dst, tp8[:])
                    else:
                        nc.vector.tensor_copy(dst, tp8[:])
                pt_map[(b, s)] = (pt_lo, pt_hi, pt_hb)

            def emit_pv(b, s, c_lo, c_hi, hcs):
                """PV matmuls for h-chunks `hcs`: mix^T into combT.

                fp8 DoubleRow over the first nk2+nk8 k-tiles (adjacent
                k-tile pairs), bf16 for the trailing nkb16.  Drain
                multiplies by the rcpb broadcast (renormalize)."""
                combT = combT_map[(b, s)]
                pt_lo, pt_hi, pt_hb = pt_map[(b, s)]
                rcpb = rcpb_map[(b, s)]
                chi_t, chb_t = c_hi
                clo_r = c_lo.rearrange("p (k h2) -> p k h2", h2=h)
                chi_r = chi_t.rearrange("p (k h2) -> p k h2", h2=h)
                plo_r = pt_lo.rearrange("p (k q) -> p k q", q=sq)
                phi_r = pt_hi.rearrange("p (k q) -> p k q", q=sq)
                for hc in hcs:
                    mm = ps_mm.tile([128, sq], F32, tag="mm",
                                    name=f"mm_{b}_{s}_{hc}")
                    for kt in range(0, nk2, 2):
                        nc.tensor.matmul(
                            mm[:],
                            clo_r[:, kt:kt + 2, hc * 128:(hc + 1) * 128],
                            plo_r[:, kt:kt + 2, :],
                            start=(kt == 0), stop=False,
                            perf_mode=mybir.MatmulPerfMode.DoubleRow)
                    for kt in range(0, nk8, 2):
                        nc.tensor.matmul(
                            mm[:],
                            chi_r[:, kt:kt + 2, hc * 128:(hc + 1) * 128],
                            phi_r[:, kt:kt + 2, :],
                            start=False, stop=False,
                            perf_mode=mybir.MatmulPerfMode.DoubleRow)
                    for kt in range(nkb16):
                        nc.tensor.matmul(
                            mm[:],
                            chb_t[:, kt * h + hc * 128:
                                  kt * h + (hc + 1) * 128],
                            pt_hb[:, kt * sq:(kt + 1) * sq],
                            start=False, stop=(kt == nkb16 - 1))
                    nc.vector.tensor_mul(
                        combT[:, hc * sq:(hc + 1) * sq], mm[:], rcpb[:])

            def emit_proj(b, s, tis):
                """Projection + tanh + store for q-tiles `tis` of super s.

                The concat's Q half is read straight from the fp16 Q^T
                tile; the mix half from combT."""
                combT = combT_map[(b, s)]
                qt_t = qt_map[(b, s)]
                for ti in tis:
                    t = s * SUPER + ti
                    ostage = out_pool.tile([128, ho], F32, tag="ostage",
                                           name=f"os_{b}_{t}")
                    for hb in range(n_hob):
                        pr = ps_mm.tile([128, hob], F32, tag="mm",
                                        name=f"pr_{b}_{t}_{hb}")
                        for dc in range(n_dc):
                            if dc < n_hc:
                                lhs = combT[:, dc * sq + ti * 128:
                                            dc * sq + (ti + 1) * 128]
                            else:
                                lhs = qt_t[:, (dc - n_hc) * sq + ti * 128:
                                           (dc - n_hc) * sq + (ti + 1) * 128]
                            nc.tensor.matmul(
                                pr[:], lhs,
                                wt_t[:, dc * ho + hb * hob:
                                     dc * ho + (hb + 1) * hob],
                                start=(dc == 0), stop=(dc == n_dc - 1))
                        nc.scalar.activation(
                            ostage[:, hb * hob:(hb + 1) * hob], pr[:],
                            mybir.ActivationFunctionType.Tanh)
                    nc.sync.dma_start(
                        out_ext[b, t * 128:(t + 1) * 128, :], ostage[:])

            # ------------------------------------------------------------
            # pipelined main program
            # ------------------------------------------------------------
            prefetched = {}   # b -> (ct_t, clo_t, chi_t); s=0 QKs pre-emitted
            for b in range(b_loc):
                if b in prefetched:
                    ct_t, clo_t, chi_t = prefetched.pop(b)
                    s0_done = True
                else:
                    # startup critical path: only qt(0) + ct stripe 0 gate
                    # the first QK matmuls (qt first -- it gates ALL of
                    # them).  clo/chi/wt (6 MB) are deferred into the
                    # first super's emission (not needed until s=1).
                    emit_qt_dma(b, 0)
                    ct_t = emit_ct_dma(b)
                    emit_const_dma()
                    clo_t = chi_t = None
                    s0_done = False

                for s in range(n_s):
                    if s + 1 < n_s:
                        emit_qt_dma(b, s + 1)
                    if s == 0 and s0_done:
                        continue
                    if s > 0:
                        row_sb = emit_rcpb_row(b, s - 1)
                        emit_rcpb_bcast(b, s - 1, row_sb)
                        combT_map[(b, s - 1)] = comb_pool.tile(
                            [128, n_hc * sq], F16, tag="comb",
                            name=f"cb_{b}_{s - 1}")
                    emit_qk_softmax(b, s, 0, ct_t)
                    if s > 0:
                        emit_pt(b, s - 1)
                        emit_pv(b, s - 1, clo_t, chi_t, range(0, 2))
                    emit_qk_softmax(b, s, 1, ct_t)
                    if s == 0 and clo_t is None:
                        clo_t, chi_t = emit_c_dma(b)
                    if s > 0:
                        emit_pv(b, s - 1, clo_t, chi_t, range(2, n_hc))
                    emit_qk_softmax(b, s, 2, ct_t)
                    if s > 0:
                        emit_proj(b, s - 1, [0, 1])
                    emit_qk_softmax(b, s, 3, ct_t)
                    if s == 0:
                        emit_wt_dma()
                    if s > 0:
                        emit_proj(b, s - 1, [2, 3])
                        pt_map.pop((b, s - 1))
                        rcpb_map.pop((b, s - 1))
                        combT_map.pop((b, s - 1))

                # --- batch tail: last super's tail stages, interleaved
                # with the next batch's prefetch DMAs and (for b+1) its
                # first QK blocks ---
                sl = n_s - 1
                row_sb = emit_rcpb_row(b, sl)
                emit_rcpb_bcast(b, sl, row_sb)
                combT_map[(b, sl)] = comb_pool.tile(
                    [128, n_hc * sq], F16, tag="comb", name=f"cb_{b}_{sl}")
                emit_pt(b, sl)
                nb = b + 1
                if nb < b_loc:
                    # WAR-safe: every reader of ct_t/qt(b,*) is emitted
                    ct_next = emit_ct_dma(nb)
                    emit_qt_dma(nb, 0)
                emit_pv(b, sl, clo_t, chi_t, range(n_hc))
                if nb < b_loc:
                    clo_n, chi_n = emit_c_dma(nb)
                    prefetched[nb] = (ct_next, clo_n, chi_n)
                    emit_qk_softmax(nb, 0, 0, ct_next)
                    emit_proj(b, sl, [0])
                    emit_qk_softmax(nb, 0, 1, ct_next)
                    emit_proj(b, sl, [1])
                    emit_qk_softmax(nb, 0, 2, ct_next)
                    emit_proj(b, sl, [2, 3])
                    emit_qk_softmax(nb, 0, 3, ct_next)
                else:
                    emit_proj(b, sl, [0, 1, 2, 3])
                pt_map.pop((b, sl))
                rcpb_map.pop((b, sl))
                combT_map.pop((b, sl))

            # mark the prefetched first-super QKs of the last batch as
            # consumed bookkeeping (handled inside the loop above via
            # p_tiles/rcp_tiles maps)

    nc.compile()
    return nc


_NC_CACHE = {}


def _get_nc(b_loc, tq, tc, h):
    key = (b_loc, tq, tc, h)
    if key not in _NC_CACHE:
        _NC_CACHE[key] = build_bass(b_loc, tq, tc, h)
    return _NC_CACHE[key]


def make_in_maps(query, context, W_attn, n_cores=N_CORES):
    b = query.shape[0]
    b_loc = b // n_cores
    tq, h = query.shape[1], query.shape[2]
    tc = context.shape[1]
    n_s = tq // (SUPER * 128)
    sq = SUPER * 128
    n_hc = h // 128
    n_kb = tc // 512
    nk2 = (tc // 128) // 2
    n_dc = 2 * h // 128
    F8NP = ml_dtypes.float8_e4m3
    BFNP = ml_dtypes.bfloat16

    q = np.ascontiguousarray(query).reshape(n_cores, b_loc, tq, h)
    c = np.ascontiguousarray(context).reshape(n_cores, b_loc, tc, h)

    # qt[i, b, s, p, hc*sq + j] = Q[i, b, s*sq + j, hc*128 + p]
    qt = np.ascontiguousarray(
        q.reshape(n_cores, b_loc, n_s, sq, n_hc, 128)
        .transpose(0, 1, 2, 5, 4, 3)
        .reshape(n_cores, b_loc, n_s, 128, n_hc * sq)
        .astype(np.float16))
    # ct[i, b, kbi, p, hc*512 + j] = C[i, b, kbi*512 + j, hc*128 + p]
    ct = np.ascontiguousarray(
        c.reshape(n_cores, b_loc, n_kb, 512, n_hc, 128)
        .transpose(0, 1, 2, 5, 4, 3)
        .reshape(n_cores, b_loc, n_kb, 128, n_hc * 512)
        .astype(np.float16))
    # c in [k-tile, h] layout: cl[i, b, p, kt*h + j] = C[i, b, kt*128+p, j]
    nk8 = nk2 - N_KT_BF16
    ckh = (c.reshape(n_cores, b_loc, 2 * nk2, 128, h)
           .transpose(0, 1, 3, 2, 4))  # [i, b, 128, 2*nk2, h]
    clo = np.ascontiguousarray(
        ckh[:, :, :, :nk2].reshape(n_cores, b_loc, 128, nk2 * h)
        .astype(F8NP))
    chi8 = np.ascontiguousarray(
        ckh[:, :, :, nk2:nk2 + nk8].reshape(n_cores, b_loc, 128, nk8 * h)
        .astype(F8NP))
    chib = np.ascontiguousarray(
        ckh[:, :, :, nk2 + nk8:].reshape(n_cores, b_loc, 128,
                                         N_KT_BF16 * h)
        .astype(BFNP))
    # wt[p, dc*h + j] = W_attn[j, dc*128 + p]
    wt = np.ascontiguousarray(
        np.ascontiguousarray(W_attn.T)
        .reshape(n_dc, 128, h).transpose(1, 0, 2)
        .reshape(128, n_dc * h).astype(np.float16))

    idf = np.eye(128, dtype=np.float32)
    idb = np.eye(128).astype(BFNP)
    ones = np.ones((1, 128), dtype=np.float32)

    in_maps = []
    for i in range(n_cores):
        in_maps.append({
            "qt": qt[i],
            "ct": ct[i],
            "clo": clo[i],
            "chi8": chi8[i],
            "chib": chib[i],
            "wt": wt,
            "idf": idf,
            "idb": idb,
            "ones": ones,
        })
    return in_maps


def kernel(query, context, W_attn, _trace=False, _trace_kwargs=None):
    b, tq, h = query.shape
    tc = context.shape[1]
    b_loc = b // N_CORES
    nc = _get_nc(b_loc, tq, tc, h)
    in_maps = make_in_maps(query, context, W_attn)
    res = run_bass_kernel_spmd(
        nc, in_maps, core_ids=list(range(N_CORES)), trace=_trace,
        **(_trace_kwargs or {}))
    out = np.concatenate([res.results[i]["out"] for i in range(N_CORES)],
                         axis=0)
    if _trace:
        return out, res
    return out


# revision 25
# speedup vs baseline: 1.4615x; 1.0013x over previous
"""Trainium2 8-core kernel for batched attention + concat projection.

Reference computation (per batch b):
    scores = Q @ C^T                  [TQ, TC]
    A      = softmax(scores, axis=-1)
    mix    = A @ C                    [TQ, H]
    out    = tanh(concat([mix, Q]) @ W^T)   [TQ, H]

Distribution: pure data-parallel over batch (B=16 across 8 cores, 2
batches per core), W replicated. No collectives needed.

v2 design (vs the f32r baseline):
  - ALL layout work is done on the host: Q^T / C^T ship pre-transposed
    (fp16), C ships pre-quantized (fp8/bf16) in [k, h] tile layout, and
    W^T ships pre-cast fp16.  The device runs zero f32 transposes and
    zero dtype-staging copies (the baseline burned ~135 us of PE time
    on f32 transposes and ~130 us of DVE on staging casts).
  - QK runs in fp16 (numerically indistinguishable from f32 here:
    sim rel-err identical to 4 decimals).  No fp32 matmuls anywhere
    means fp16/bf16 LDWEIGHTS keep FWL and hide behind the matmul
    stream, where the baseline's fp32 QK was LDWEIGHTS-paced
    (289 ns/MM for a 216 ns ideal).
  - SUPER=4 (512 q columns per super-iteration) so the fp8 DoubleRow
    PV matmuls run at free-dim 512, where DR's ~2x rate is not
    LDW-limited (baseline FD=256 measured 157 ns per 2-k-tile pair;
    FD=512 target ~224 ns per pair covering 2x the columns).
  - softmax over free axis k: DVE reduce_max(negate) -> ACT exp with
    per-partition bias, bf16 output (unnormalized, max ~= 1) and f32
    row-sum accumulator -> DVE reciprocal.  Normalization is folded
    into the PV PSUM drain via a [128, sq] broadcast of 1/rowsum
    (PE: transpose rcp columns to a row + ones outer-product).
  - P^T via bf16 PE transposes packed 8-per-PSUM-bank; drains cast to
    the per-half PV dtype (fp8 lo always; hi fp8 on batch 0, bf16 on
    batch 1 -- same error budget as the baseline, measured 1.74e-2
    against the 2e-2 gate).
  - proj reads the concat's Q half straight out of the fp16 Q^T tile
    (no comb assembly for that half) and runs fp16 x fp16 -> tanh.
  - Stages of super s-1 are interleaved between the QK blocks of
    super s so the in-order PE stream always has ready work while the
    softmax chain (DVE reduce_max -> ACT exp) of the current q-tile
    completes; scores PSUM is WAR-recycled per q-tile.
"""

import numpy as np
import ml_dtypes

import concourse.bacc as bacc
import concourse.tile as tile
import concourse.mybir as mybir
from concourse.bass_utils import run_bass_kernel_spmd

F32 = mybir.dt.float32
F32R = mybir.dt.float32r
F16 = mybir.dt.float16
BF16 = mybir.dt.bfloat16
FP8 = mybir.dt.float8e4

N_CORES = 8
B, TQ, TC, H = 16, 2048, 2048, 1024

SUPER = 4              # q-tiles per super-iteration
PG = 8                 # bf16 transposes packed per PSUM bank


# PV error is dominated by the fp8 quantization of C (P-fp8 alone costs
# only ~0.3%): sim err ~= 0.0176 * sqrt(fp8_fraction), HW ~= 1.14x sim.
# Keep the last N_KT_BF16 k-tiles of every batch in bf16 (f = 7/8):
# predicted HW rel err ~0.0187 against the 2e-2 gate.
N_KT_BF16 = 2

# DoubleRowSwInterleave: ship the fp8 C stationary chunks pre-interleaved
# ([A127, B127, A126, B126, ...] per partition row -- the PE's native DR
# weight read order) so LDWEIGHTS walks memory linearly instead of the
# strided two-tile interleave (plain DR LDW measured 162 ns and exposes
# ~47 ns/MM past the 216 ns DR matmul).
SW_INTERLEAVE = True


def build_bass(b_loc, tq, tc, h, n_cores=N_CORES):
    """Build the per-core Bass graph. All cores run the same graph (SPMD)."""
    d = 2 * h
    ho = h
    n_qt = tq // 128       # q tiles
    n_kt = tc // 128       # k tiles
    n_hc = h // 128        # h chunks
    n_dc = d // 128        # d chunks (contraction for proj)
    kb = 512               # QK rhs block
    n_kb = tc // kb
    hob = 512              # proj output block
    n_hob = ho // hob
    assert n_qt % SUPER == 0
    n_s = n_qt // SUPER
    sq = SUPER * 128       # q columns per super-iteration
    nk2 = n_kt // 2
    nkb16 = N_KT_BF16      # trailing k-tiles of the hi half kept bf16
    nk8 = nk2 - nkb16      # fp8 k-tiles in the hi half
    assert nk8 % 2 == 0 and nkb16 % 2 == 0

    nc = bacc.Bacc("TRN2", target_bir_lowering=False, debug=False,
                   num_devices=n_cores)

    # host-prepped inputs (see make_in_maps for layouts)
    qt_ext = nc.declare_dram_parameter("qt", [b_loc, n_s, 128, n_hc * sq],
                                       F16, isOutput=False)
    ct_ext = nc.declare_dram_parameter("ct", [b_loc, n_kb, 128, n_hc * kb],
                                       F16, isOutput=False)
    clo_ext = nc.declare_dram_parameter("clo", [b_loc, 128, nk2 * h], FP8,
                                        isOutput=False)
    chi8_ext = nc.declare_dram_parameter("chi8", [b_loc, 128, nk8 * h], FP8,
                                         isOutput=False)
    chib_ext = nc.declare_dram_parameter("chib", [b_loc, 128, nkb16 * h],
                                         BF16, isOutput=False)
    wt_ext = nc.declare_dram_parameter("wt", [128, n_dc * ho], F16,
                                       isOutput=False)
    idf_ext = nc.declare_dram_parameter("idf", [128, 128], F32, isOutput=False)
    idb_ext = nc.declare_dram_parameter("idb", [128, 128], BF16, isOutput=False)
    ones_ext = nc.declare_dram_parameter("ones", [1, 128], F32R, isOutput=False)
    out_ext = nc.declare_dram_parameter("out", [b_loc, tq, ho], F32,
                                        isOutput=True)

    with tile.TileContext(nc) as tc_:
        with (
            tc_.tile_pool(name="const", bufs=1) as const_pool,
            tc_.tile_pool(name="ct", bufs=1) as ct_pool,
            tc_.tile_pool(name="qt", bufs=3) as qt_pool,
            tc_.tile_pool(name="clo", bufs=1) as clo_pool,
            tc_.tile_pool(name="chi8", bufs=1) as chi8_pool,
            tc_.tile_pool(name="chib", bufs=1) as chib_pool,
            tc_.tile_pool(name="p", bufs=6) as p_pool,
            tc_.tile_pool(name="ptlo", bufs=1) as ptlo_pool,
            tc_.tile_pool(name="pthi8", bufs=1) as pthi8_pool,
            tc_.tile_pool(name="pthib", bufs=1) as pthib_pool,
            tc_.tile_pool(name="comb", bufs=2) as comb_pool,
            tc_.tile_pool(name="ostage", bufs=3) as out_pool,
            tc_.tile_pool(name="stats", bufs=24) as stats_pool,
            tc_.tile_pool(name="rrow", bufs=2) as rrow_pool,
            tc_.tile_pool(name="rcpb", bufs=2) as rcpb_pool,
            tc_.tile_pool(name="ps_s", bufs=4, space="PSUM") as ps_s,
            tc_.tile_pool(name="ps_tp", bufs=2, space="PSUM") as ps_tp,
            tc_.tile_pool(name="ps_mm", bufs=2, space="PSUM") as ps_mm,
        ):
            # --- constants (wt DMA deferred off the startup critical path) ---
            idf = const_pool.tile([128, 128], F32, tag="idf")
            idb = const_pool.tile([128, 128], BF16, tag="idb")
            ones_r = const_pool.tile([1, 128], F32R, tag="ones")
            wt_t = const_pool.tile([128, n_dc * ho], F16, tag="wt")

            def emit_const_dma():
                nc.sync.dma_start(idf[:], idf_ext[:])
                nc.sync.dma_start(idb[:], idb_ext[:])
                nc.sync.dma_start(ones_r[:], ones_ext[:])

            def emit_wt_dma():
                for half in range(2):
                    nc.sync.dma_start(
                        wt_t[:, half * 8 * ho:(half + 1) * 8 * ho],
                        wt_ext[:, half * 8 * ho:(half + 1) * 8 * ho])

            p_tiles = {}      # (b, t) -> unnormalized bf16 P tile
            rcp_tiles = {}    # (b, t) -> [128, 1] reciprocal row sums
            qt_map = {}       # (b, s) -> fp16 Q^T tile (QK lhs + proj Q-half)
            combT_map = {}    # (b, s) -> fp16 mix^T tile
            pt_map = {}       # (b, s) -> (pt_lo, pt_hi)
            rcpb_map = {}     # (b, s) -> [128, sq] broadcast reciprocal

            def emit_qt_dma(b, s):
                qt_t = qt_pool.tile([128, n_hc * sq], F16, tag="qt",
                                    name=f"qt_{b}_{s}")
                nc.sync.dma_start(qt_t[:], qt_ext[b, s])
                qt_map[(b, s)] = qt_t

            def emit_ct_dma(b):
                ct_t = ct_pool.tile([128, n_kb * n_hc * kb], F16, tag="ct",
                                    name=f"ct_{b}")
                for kbi in range(n_kb):
                    nc.sync.dma_start(
                        ct_t[:, kbi * n_hc * kb:(kbi + 1) * n_hc * kb],
                        ct_ext[b, kbi])
                return ct_t

            def emit_c_dma(b):
                clo_t = clo_pool.tile([128, nk2 * h], FP8, tag="clo",
                                      name=f"clo_{b}")
                nc.sync.dma_start(clo_t[:], clo_ext[b])
                chi_t = chi8_pool.tile([128, nk8 * h], FP8, tag="chi8",
                                       name=f"chi8_{b}")
                nc.sync.dma_start(chi_t[:], chi8_ext[b])
                chb_t = chib_pool.tile([128, nkb16 * h], BF16, tag="chib",
                                       name=f"chib_{b}")
                nc.sync.dma_start(chb_t[:], chib_ext[b])
                return clo_t, (chi_t, chb_t)

            def emit_qk_softmax(b, s, ti, ct_t):
                """Scores for q-tile (s, ti) + softmax chain.

                kbi-outer: each 512-col PSUM bank finishes its hc
                accumulation early, so its partial row-max runs on DVE
                while the next bank's matmuls stream, and each exp
                releases its bank for the next q-tile's QK (per-bank
                tiles from a bufs=4 pool) with ~1.5 us latency instead
                of a 4.4 us whole-tile WAR.

                exp output is the UNNORMALIZED bf16 P (max ~= 1); the
                row-sums of exact exp values accumulate into l_tot and
                the reciprocal is kept for the PV-drain renormalize."""
                t = s * SUPER + ti
                qt_t = qt_map[(b, s)]
                pm = stats_pool.tile([128, n_kb], F32, tag="pm",
                                     name=f"pm_{b}_{t}")
                banks = []
                for kbi in range(n_kb):
                    sb = ps_s.tile([128, kb], F32, tag="s",
                                   name=f"s_{b}_{t}_{kbi}")
                    for hc in range(n_hc):
                        lhs = qt_t[:, hc * sq + ti * 128:
                                   hc * sq + (ti + 1) * 128]
                        rhs = ct_t[:, kbi * n_hc * kb + hc * kb:
                                   kbi * n_hc * kb + (hc + 1) * kb]
                        nc.tensor.matmul(
                            sb[:], lhs, rhs,
                            start=(hc == 0), stop=(hc == n_hc - 1))
                    nc.vector.reduce_max(
                        pm[:, kbi:kbi + 1], sb[:], axis=mybir.AxisListType.X)
                    banks.append(sb)
                negm = stats_pool.tile([128, 1], F32, tag="negm",
                                       name=f"negm_{b}_{t}")
                nc.vector.reduce_max(
                    negm[:], pm[:], axis=mybir.AxisListType.X, negate=True)
                lacc = stats_pool.tile([128, n_kb], F32, tag="lacc",
                                       name=f"lacc_{b}_{t}")
                nc.vector.memset(lacc[:], 0.0)
                p = p_pool.tile([128, tc], BF16, tag="p", name=f"p_{b}_{t}")
                for kbi, sb in enumerate(banks):
                    nc.scalar.activation(
                        p[:, kbi * kb:(kbi + 1) * kb], sb[:],
                        mybir.ActivationFunctionType.Exp,
                        bias=negm[:], scale=1.0,
                        accum_out=lacc[:, kbi:kbi + 1])
                l_tot = stats_pool.tile([128, 1], F32, tag="ltot",
                                        name=f"lt_{b}_{t}")
                nc.vector.reduce_sum(l_tot[:], lacc[:],
                                     axis=mybir.AxisListType.X)
                rcp = stats_pool.tile([128, 1], F32, tag="rcp",
                                      name=f"rcp_{b}_{t}")
                nc.vector.reciprocal(rcp[:], l_tot[:])
                p_tiles[(b, t)] = p
                rcp_tiles[(b, t)] = rcp

            def emit_rcpb_row(b, s):
                """Transpose the SUPER rcp [128,1] columns into one row."""
                row_ps = ps_tp.tile([128, sq], F32, tag="tp",
                                    name=f"rrow_{b}_{s}")
                for ti in range(SUPER):
                    rcp = rcp_tiles.pop((b, s * SUPER + ti))
                    nc.tensor.transpose(
                        row_ps[0:1, ti * 128:(ti + 1) * 128], rcp[:], idf[:])
                row_sb = rrow_pool.tile([1, sq], F32R, tag="rrow",
                                        name=f"rrs_{b}_{s}")
                nc.scalar.copy(row_sb[:], row_ps[0:1, 0:sq])
                return row_sb

            def emit_rcpb_bcast(b, s, row_sb):
                """Ones outer-product broadcast of 1/rowsum to [128, sq]."""
                bc_ps = ps_mm.tile([128, sq], F32, tag="mm",
                                   name=f"rbc_{b}_{s}")
                nc.tensor.matmul(bc_ps[:], ones_r[:], row_sb[:],
                                 start=True, stop=True)
                rcpb = rcpb_pool.tile([128, sq], F32, tag="rcpb",
                                      name=f"rcpb_{b}_{s}")
                nc.vector.tensor_copy(rcpb[:], bc_ps[:])
                rcpb_map[(b, s)] = rcpb

            def emit_pt(b, s):
                """P^T for super s: bf16 PE transposes packed into PSUM
                banks, drained by wide ACT/DVE copies casting each
                k-half to its PV dtype."""
                pt_lo = ptlo_pool.tile([128, nk2 * sq], FP8, tag="ptlo",
                                       name=f"ptlo_{b}_{s}")
                pt_hi = pthi8_pool.tile([128, nk8 * sq], FP8,
                                        tag="pthi8", name=f"pthi_{b}_{s}")
                pt_hb = pthib_pool.tile([128, nkb16 * sq], BF16,
                                        tag="pthib", name=f"pthb_{b}_{s}")
                ps = [p_tiles.pop((b, s * SUPER + ti)) for ti in range(SUPER)]
                # pack one k-tile PAIR x all SUPER q-tiles per PSUM bank so
                # the drain is a single fully-CONTIGUOUS [128, 2*sq] copy
                # into the [k, q] pt layout (strided drains measured 2x
                # slower and made P^T drain-paced).
                for kp in range(n_kt // 2):
                    if kp < nk2 // 2:
                        tgt, kbase = pt_lo, 0
                    elif kp < (nk2 + nk8) // 2:
                        tgt, kbase = pt_hi, nk2
                    else:
                        tgt, kbase = pt_hb, nk2 + nk8
                    tp8 = ps_tp.tile([128, 2 * sq], BF16, tag="tp",
                                     name=f"tp8_{b}_{s}_{kp}")
                    for j in range(2):
                        kt = kp * 2 + j
                        for ti in range(SUPER):
                            nc.tensor.transpose(
                                tp8[:, (j * SUPER + ti) * 128:
                                    (j * SUPER + ti + 1) * 128],
                                ps[ti][:, kt * 128:(kt + 1) * 128],
                                idb[:])
                    dst = tgt[:, (kp * 2 - kbase) * sq:
                              (kp * 2 - kbase + 2) * sq]
                    if kp % 2 == 0:
                        nc.scalar.copy(dst, tp8[:])
                    else:
                        nc.vector.tensor_copy(dst, tp8[:])
                pt_map[(b, s)] = (pt_lo, pt_hi, pt_hb)

            def emit_pv(b, s, c_lo, c_hi, hcs):
                """PV matmuls for h-chunks `hcs`: mix^T into combT.

                fp8 DoubleRow over the first nk2+nk8 k-tiles (adjacent
                k-tile pairs), bf16 for the trailing nkb16.  Drain
                multiplies by the rcpb broadcast (renormalize)."""
                combT = combT_map[(b, s)]
                pt_lo, pt_hi, pt_hb = pt_map[(b, s)]
                rcpb = rcpb_map[(b, s)]
                chi_t, chb_t = c_hi
                plo_r = pt_lo.rearrange("p (k q) -> p k q", q=sq)
                phi_r = pt_hi.rearrange("p (k q) -> p k q", q=sq)
                def dr_lhs(c_t, pk, hc):
                    if SW_INTERLEAVE:
                        # [pair, hc, 2x128 interleaved-reversed] layout
                        off = pk * 2 * h + hc * 256
                        return c_t[:, off:off + 256].rearrange(
                            "p (two c) -> p two c", two=2)
                    r = c_t.rearrange("p (k h2) -> p k h2", h2=h)
                    return r[:, 2 * pk:2 * pk + 2, hc * 128:(hc + 1) * 128]

                dr_mode = (mybir.MatmulPerfMode.DoubleRowSwInterleave
                           if SW_INTERLEAVE else
                           mybir.MatmulPerfMode.DoubleRow)
                for hc in hcs:
                    mm = ps_mm.tile([128, sq], F32, tag="mm",
                                    name=f"mm_{b}_{s}_{hc}")
                    for kt in range(0, nk2, 2):
                        nc.tensor.matmul(
                            mm[:],
                            dr_lhs(c_lo, kt // 2, hc),
                            plo_r[:, kt:kt + 2, :],
                            start=(kt == 0), stop=False,
                            perf_mode=dr_mode)
                    for kt in range(0, nk8, 2):
                        nc.tensor.matmul(
                            mm[:],
                            dr_lhs(chi_t, kt // 2, hc),
                            phi_r[:, kt:kt + 2, :],
                            start=False, stop=False,
                            perf_mode=dr_mode)
                    for kt in range(nkb16):
                        nc.tensor.matmul(
                            mm[:],
                            chb_t[:, kt * h + hc * 128:
                                  kt * h + (hc + 1) * 128],
                            pt_hb[:, kt * sq:(kt + 1) * sq],
                            start=False, stop=(kt == nkb16 - 1))
                    nc.vector.tensor_mul(
                        combT[:, hc * sq:(hc + 1) * sq], mm[:], rcpb[:])

            def emit_proj(b, s, tis):
                """Projection + tanh + store for q-tiles `tis` of super s.

                The concat's Q half is read straight from the fp16 Q^T
                tile; the mix half from combT."""
                combT = combT_map[(b, s)]
                qt_t = qt_map[(b, s)]
                for ti in tis:
                    t = s * SUPER + ti
                    ostage = out_pool.tile([128, ho], F32, tag="ostage",
                                           name=f"os_{b}_{t}")
                    for hb in range(n_hob):
                        pr = ps_mm.tile([128, hob], F32, tag="mm",
                                        name=f"pr_{b}_{t}_{hb}")
                        for dc in range(n_dc):
                            if dc < n_hc:
                                lhs = combT[:, dc * sq + ti * 128:
                                            dc * sq + (ti + 1) * 128]
                            else:
                                lhs = qt_t[:, (dc - n_hc) * sq + ti * 128:
                                           (dc - n_hc) * sq + (ti + 1) * 128]
                            nc.tensor.matmul(
                                pr[:], lhs,
                                wt_t[:, dc * ho + hb * hob:
                                     dc * ho + (hb + 1) * hob],
                                start=(dc == 0), stop=(dc == n_dc - 1))
                        nc.scalar.activation(
                            ostage[:, hb * hob:(hb + 1) * hob], pr[:],
                            mybir.ActivationFunctionType.Tanh)
                    nc.sync.dma_start(
                        out_ext[b, t * 128:(t + 1) * 128, :], ostage[:])

            # ------------------------------------------------------------
            # pipelined main program
            # ------------------------------------------------------------
            prefetched = {}   # b -> (ct_t, clo_t, chi_t); s=0 QKs pre-emitted
            for b in range(b_loc):
                if b in prefetched:
                    ct_t, clo_t, chi_t = prefetched.pop(b)
                    s0_done = True
                else:
                    # startup critical path: only qt(0) + ct stripe 0 gate
                    # the first QK matmuls (qt first -- it gates ALL of
                    # them).  clo/chi/wt (6 MB) are deferred into the
                    # first super's emission (not needed until s=1).
                    emit_qt_dma(b, 0)
                    ct_t = emit_ct_dma(b)
                    emit_const_dma()
                    clo_t = chi_t = None
                    s0_done = False

                for s in range(n_s):
                    if s + 1 < n_s:
                        emit_qt_dma(b, s + 1)
                    if s == 0 and s0_done:
                        continue
                    if s > 0:
                        row_sb = emit_rcpb_row(b, s - 1)
                        emit_rcpb_bcast(b, s - 1, row_sb)
                        combT_map[(b, s - 1)] = comb_pool.tile(
                            [128, n_hc * sq], F16, tag="comb",
                            name=f"cb_{b}_{s - 1}")
                    emit_qk_softmax(b, s, 0, ct_t)
                    if s > 0:
                        emit_pt(b, s - 1)
                        emit_pv(b, s - 1, clo_t, chi_t, range(0, 2))
                    emit_qk_softmax(b, s, 1, ct_t)
                    if s == 0 and clo_t is None:
                        clo_t, chi_t = emit_c_dma(b)
                    if s > 0:
                        emit_pv(b, s - 1, clo_t, chi_t, range(2, n_hc))
                    emit_qk_softmax(b, s, 2, ct_t)
                    if s > 0:
                        emit_proj(b, s - 1, [0, 1])
                    emit_qk_softmax(b, s, 3, ct_t)
                    if s == 0:
                        emit_wt_dma()
                    if s > 0:
                        emit_proj(b, s - 1, [2, 3])
                        pt_map.pop((b, s - 1))
                        rcpb_map.pop((b, s - 1))
                        combT_map.pop((b, s - 1))

                # --- batch tail: last super's tail stages, interleaved
                # with the next batch's prefetch DMAs and (for b+1) its
                # first QK blocks ---
                sl = n_s - 1
                row_sb = emit_rcpb_row(b, sl)
                emit_rcpb_bcast(b, sl, row_sb)
                combT_map[(b, sl)] = comb_pool.tile(
                    [128, n_hc * sq], F16, tag="comb", name=f"cb_{b}_{sl}")
                emit_pt(b, sl)
                nb = b + 1
                if nb < b_loc:
                    # WAR-safe: every reader of ct_t/qt(b,*) is emitted
                    ct_next = emit_ct_dma(nb)
                    emit_qt_dma(nb, 0)
                emit_pv(b, sl, clo_t, chi_t, range(n_hc))
                if nb < b_loc:
                    clo_n, chi_n = emit_c_dma(nb)
                    prefetched[nb] = (ct_next, clo_n, chi_n)
                    emit_qk_softmax(nb, 0, 0, ct_next)
                    emit_proj(b, sl, [0])
                    emit_qk_softmax(nb, 0, 1, ct_next)
                    emit_proj(b, sl, [1])
                    emit_qk_softmax(nb, 0, 2, ct_next)
                    emit_proj(b, sl, [2, 3])
                    emit_qk_softmax(nb, 0, 3, ct_next)
                else:
                    emit_proj(b, sl, [0, 1, 2, 3])
                pt_map.pop((b, sl))
                rcpb_map.pop((b, sl))
                combT_map.pop((b, sl))

            # mark the prefetched first-super QKs of the last batch as
            # consumed bookkeeping (handled inside the loop above via
            # p_tiles/rcp_tiles maps)

    nc.compile()
    return nc


_NC_CACHE = {}


def _get_nc(b_loc, tq, tc, h):
    key = (b_loc, tq, tc, h)
    if key not in _NC_CACHE:
        _NC_CACHE[key] = build_bass(b_loc, tq, tc, h)
    return _NC_CACHE[key]


def make_in_maps(query, context, W_attn, n_cores=N_CORES):
    b = query.shape[0]
    b_loc = b // n_cores
    tq, h = query.shape[1], query.shape[2]
    tc = context.shape[1]
    n_s = tq // (SUPER * 128)
    sq = SUPER * 128
    n_hc = h // 128
    n_kb = tc // 512
    nk2 = (tc // 128) // 2
    n_dc = 2 * h // 128
    F8NP = ml_dtypes.float8_e4m3
    BFNP = ml_dtypes.bfloat16

    q = np.ascontiguousarray(query).reshape(n_cores, b_loc, tq, h)
    c = np.ascontiguousarray(context).reshape(n_cores, b_loc, tc, h)

    # qt[i, b, s, p, hc*sq + j] = Q[i, b, s*sq + j, hc*128 + p]
    qt = np.ascontiguousarray(
        q.reshape(n_cores, b_loc, n_s, sq, n_hc, 128)
        .transpose(0, 1, 2, 5, 4, 3)
        .reshape(n_cores, b_loc, n_s, 128, n_hc * sq)
        .astype(np.float16))
    # ct[i, b, kbi, p, hc*512 + j] = C[i, b, kbi*512 + j, hc*128 + p]
    ct = np.ascontiguousarray(
        c.reshape(n_cores, b_loc, n_kb, 512, n_hc, 128)
        .transpose(0, 1, 2, 5, 4, 3)
        .reshape(n_cores, b_loc, n_kb, 128, n_hc * 512)
        .astype(np.float16))
    # c in [k-tile, h] layout: cl[i, b, p, kt*h + j] = C[i, b, kt*128+p, j]
    nk8 = nk2 - N_KT_BF16
    ckh = (c.reshape(n_cores, b_loc, 2 * nk2, 128, h)
           .transpose(0, 1, 3, 2, 4))  # [i, b, 128, 2*nk2, h]

    def fp8_pairs(kt0, kt1):
        """fp8 stationary chunks for k-tile pairs [kt0, kt1)."""
        npair = (kt1 - kt0) // 2
        blk = ckh[:, :, :, kt0:kt1].astype(F8NP)
        if not SW_INTERLEAVE:
            return np.ascontiguousarray(
                blk.reshape(n_cores, b_loc, 128, (kt1 - kt0) * h))
        # [i, b, p, pair, hc, j, ko] with j reversed within each hc
        # chunk and the pair's two k-tiles interleaved: the PE's native
        # DoubleRow weight read order, stored contiguously.
        a = blk.reshape(n_cores, b_loc, 128, npair, 2, n_hc, 128)
        a = a[:, :, :, :, :, :, ::-1]          # reverse j
        a = a.transpose(0, 1, 2, 3, 5, 6, 4)   # [..., pair, hc, j, ko]
        return np.ascontiguousarray(
            a.reshape(n_cores, b_loc, 128, npair * 2 * h))

    clo = fp8_pairs(0, nk2)
    chi8 = fp8_pairs(nk2, nk2 + nk8)
    chib = np.ascontiguousarray(
        ckh[:, :, :, nk2 + nk8:].reshape(n_cores, b_loc, 128,
                                         N_KT_BF16 * h)
        .astype(BFNP))
    # wt[p, dc*h + j] = W_attn[j, dc*128 + p]
    wt = np.ascontiguousarray(
        np.ascontiguousarray(W_attn.T)
        .reshape(n_dc, 128, h).transpose(1, 0, 2)
        .reshape(128, n_dc * h).astype(np.float16))

    idf = np.eye(128, dtype=np.float32)
    idb = np.eye(128).astype(BFNP)
    ones = np.ones((1, 128), dtype=np.float32)

    in_maps = []
    for i in range(n_cores):
        in_maps.append({
            "qt": qt[i],
            "ct": ct[i],
            "clo": clo[i],
            "chi8": chi8[i],
            "chib": chib[i],
            "wt": wt,
            "idf": idf,
            "idb": idb,
            "ones": ones,
        })
    return in_maps


def kernel(query, context, W_attn, _trace=False, _trace_kwargs=None):
    b, tq, h = query.shape
    tc = context.shape[1]
    b_loc = b // N_CORES
    nc = _get_nc(b_loc, tq, tc, h)
    in_maps = make_in_maps(query, context, W_attn)
    res = run_bass_kernel_spmd(
        nc, in_maps, core_ids=list(range(N_CORES)), trace=_trace,
        **(_trace_kwargs or {}))
    out = np.concatenate([res.results[i]["out"] for i in range(N_CORES)],
                         axis=0)
    if _trace:
        return out, res
    return out


# revision 27
# speedup vs baseline: 1.4668x; 1.0036x over previous
"""Trainium2 8-core kernel for batched attention + concat projection.

Reference computation (per batch b):
    scores = Q @ C^T                  [TQ, TC]
    A      = softmax(scores, axis=-1)
    mix    = A @ C                    [TQ, H]
    out    = tanh(concat([mix, Q]) @ W^T)   [TQ, H]

Distribution: pure data-parallel over batch (B=16 across 8 cores, 2
batches per core), W replicated. No collectives needed.

v2 design (vs the f32r baseline):
  - ALL layout work is done on the host: Q^T / C^T ship pre-transposed
    (fp16), C ships pre-quantized (fp8/bf16) in [k, h] tile layout, and
    W^T ships pre-cast fp16.  The device runs zero f32 transposes and
    zero dtype-staging copies (the baseline burned ~135 us of PE time
    on f32 transposes and ~130 us of DVE on staging casts).
  - QK runs in fp16 (numerically indistinguishable from f32 here:
    sim rel-err identical to 4 decimals).  No fp32 matmuls anywhere
    means fp16/bf16 LDWEIGHTS keep FWL and hide behind the matmul
    stream, where the baseline's fp32 QK was LDWEIGHTS-paced
    (289 ns/MM for a 216 ns ideal).
  - SUPER=4 (512 q columns per super-iteration) so the fp8 DoubleRow
    PV matmuls run at free-dim 512, where DR's ~2x rate is not
    LDW-limited (baseline FD=256 measured 157 ns per 2-k-tile pair;
    FD=512 target ~224 ns per pair covering 2x the columns).
  - softmax over free axis k: DVE reduce_max(negate) -> ACT exp with
    per-partition bias, bf16 output (unnormalized, max ~= 1) and f32
    row-sum accumulator -> DVE reciprocal.  Normalization is folded
    into the PV PSUM drain via a [128, sq] broadcast of 1/rowsum
    (PE: transpose rcp columns to a row + ones outer-product).
  - P^T via bf16 PE transposes packed 8-per-PSUM-bank; drains cast to
    the per-half PV dtype (fp8 lo always; hi fp8 on batch 0, bf16 on
    batch 1 -- same error budget as the baseline, measured 1.74e-2
    against the 2e-2 gate).
  - proj reads the concat's Q half straight out of the fp16 Q^T tile
    (no comb assembly for that half) and runs fp16 x fp16 -> tanh.
  - Stages of super s-1 are interleaved between the QK blocks of
    super s so the in-order PE stream always has ready work while the
    softmax chain (DVE reduce_max -> ACT exp) of the current q-tile
    completes; scores PSUM is WAR-recycled per q-tile.
"""

import numpy as np
import ml_dtypes

import concourse.bacc as bacc
import concourse.tile as tile
import concourse.mybir as mybir
from concourse.bass_utils import run_bass_kernel_spmd

F32 = mybir.dt.float32
F32R = mybir.dt.float32r
F16 = mybir.dt.float16
BF16 = mybir.dt.bfloat16
FP8 = mybir.dt.float8e4

N_CORES = 8
B, TQ, TC, H = 16, 2048, 2048, 1024

SUPER = 4              # q-tiles per super-iteration
PG = 8                 # bf16 transposes packed per PSUM bank


# PV error is dominated by the fp8 quantization of C (P-fp8 alone costs
# only ~0.3%): sim err ~= 0.0176 * sqrt(fp8_fraction), HW ~= 1.14x sim.
# Keep the last N_KT_BF16 k-tiles of every batch in bf16 (f = 7/8):
# predicted HW rel err ~0.0187 against the 2e-2 gate.
N_KT_BF16 = 2

# DoubleRowSwInterleave: ship the fp8 C stationary chunks pre-interleaved
# ([A127, B127, A126, B126, ...] per partition row -- the PE's native DR
# weight read order) so LDWEIGHTS walks memory linearly instead of the
# strided two-tile interleave (plain DR LDW measured 162 ns and exposes
# ~47 ns/MM past the 216 ns DR matmul).
SW_INTERLEAVE = True


def build_bass(b_loc, tq, tc, h, n_cores=N_CORES):
    """Build the per-core Bass graph. All cores run the same graph (SPMD)."""
    d = 2 * h
    ho = h
    n_qt = tq // 128       # q tiles
    n_kt = tc // 128       # k tiles
    n_hc = h // 128        # h chunks
    n_dc = d // 128        # d chunks (contraction for proj)
    kb = 512               # QK rhs block
    n_kb = tc // kb
    hob = 512              # proj output block
    n_hob = ho // hob
    assert n_qt % SUPER == 0
    n_s = n_qt // SUPER
    sq = SUPER * 128       # q columns per super-iteration
    nk2 = n_kt // 2
    nkb16 = N_KT_BF16      # trailing k-tiles of the hi half kept bf16
    nk8 = nk2 - nkb16      # fp8 k-tiles in the hi half
    assert nk8 % 2 == 0 and nkb16 % 2 == 0

    nc = bacc.Bacc("TRN2", target_bir_lowering=False, debug=False,
                   num_devices=n_cores)

    # host-prepped inputs (see make_in_maps for layouts)
    qt_ext = nc.declare_dram_parameter("qt", [b_loc, n_s, 128, n_hc * sq],
                                       F16, isOutput=False)
    ct_ext = nc.declare_dram_parameter("ct", [b_loc, n_kb, 128, n_hc * kb],
                                       F16, isOutput=False)
    clo_ext = nc.declare_dram_parameter("clo", [b_loc, 128, nk2 * h], FP8,
                                        isOutput=False)
    chi8_ext = nc.declare_dram_parameter("chi8", [b_loc, 128, nk8 * h], FP8,
                                         isOutput=False)
    chib_ext = nc.declare_dram_parameter("chib", [b_loc, 128, nkb16 * h],
                                         BF16, isOutput=False)
    wt_ext = nc.declare_dram_parameter("wt", [128, n_dc * ho], F16,
                                       isOutput=False)
    idf_ext = nc.declare_dram_parameter("idf", [128, 128], F32, isOutput=False)
    idb_ext = nc.declare_dram_parameter("idb", [128, 128], BF16, isOutput=False)
    ones_ext = nc.declare_dram_parameter("ones", [1, 128], F32R, isOutput=False)
    out_ext = nc.declare_dram_parameter("out", [b_loc, tq, ho], F32,
                                        isOutput=True)

    with tile.TileContext(nc) as tc_:
        with (
            tc_.tile_pool(name="const", bufs=1) as const_pool,
            tc_.tile_pool(name="ct", bufs=1) as ct_pool,
            tc_.tile_pool(name="qt", bufs=3) as qt_pool,
            tc_.tile_pool(name="clo", bufs=1) as clo_pool,
            tc_.tile_pool(name="chi8", bufs=1) as chi8_pool,
            tc_.tile_pool(name="chib", bufs=1) as chib_pool,
            tc_.tile_pool(name="p", bufs=6) as p_pool,
            tc_.tile_pool(name="ptlo", bufs=1) as ptlo_pool,
            tc_.tile_pool(name="pthi8", bufs=1) as pthi8_pool,
            tc_.tile_pool(name="pthib", bufs=1) as pthib_pool,
            tc_.tile_pool(name="comb", bufs=2) as comb_pool,
            tc_.tile_pool(name="ostage", bufs=3) as out_pool,
            tc_.tile_pool(name="stats", bufs=24) as stats_pool,
            tc_.tile_pool(name="rrow", bufs=2) as rrow_pool,
            tc_.tile_pool(name="rcpb", bufs=2) as rcpb_pool,
            tc_.tile_pool(name="ps_s", bufs=4, space="PSUM") as ps_s,
            tc_.tile_pool(name="ps_tp", bufs=2, space="PSUM") as ps_tp,
            tc_.tile_pool(name="ps_mm", bufs=2, space="PSUM") as ps_mm,
        ):
            # --- constants (wt DMA deferred off the startup critical path) ---
            idf = const_pool.tile([128, 128], F32, tag="idf")
            idb = const_pool.tile([128, 128], BF16, tag="idb")
            ones_r = const_pool.tile([1, 128], F32R, tag="ones")
            wt_t = const_pool.tile([128, n_dc * ho], F16, tag="wt")

            def emit_const_dma():
                nc.sync.dma_start(idf[:], idf_ext[:])
                nc.sync.dma_start(idb[:], idb_ext[:])
                nc.sync.dma_start(ones_r[:], ones_ext[:])

            def emit_wt_dma():
                for half in range(2):
                    nc.sync.dma_start(
                        wt_t[:, half * 8 * ho:(half + 1) * 8 * ho],
                        wt_ext[:, half * 8 * ho:(half + 1) * 8 * ho])

            p_tiles = {}      # (b, t) -> unnormalized bf16 P tile
            rcp_tiles = {}    # (b, t) -> [128, 1] reciprocal row sums
            qt_map = {}       # (b, s) -> fp16 Q^T tile (QK lhs + proj Q-half)
            combT_map = {}    # (b, s) -> fp16 mix^T tile
            pt_map = {}       # (b, s) -> (pt_lo, pt_hi)
            rcpb_map = {}     # (b, s) -> [128, sq] broadcast reciprocal

            def emit_qt_dma(b, s):
                qt_t = qt_pool.tile([128, n_hc * sq], F16, tag="qt",
                                    name=f"qt_{b}_{s}")
                nc.sync.dma_start(qt_t[:], qt_ext[b, s])
                qt_map[(b, s)] = qt_t

            def emit_ct_dma(b):
                ct_t = ct_pool.tile([128, n_kb * n_hc * kb], F16, tag="ct",
                                    name=f"ct_{b}")
                for kbi in range(n_kb):
                    nc.sync.dma_start(
                        ct_t[:, kbi * n_hc * kb:(kbi + 1) * n_hc * kb],
                        ct_ext[b, kbi])
                return ct_t

            def emit_c_dma(b):
                clo_t = clo_pool.tile([128, nk2 * h], FP8, tag="clo",
                                      name=f"clo_{b}")
                nc.sync.dma_start(clo_t[:], clo_ext[b])
                chi_t = chi8_pool.tile([128, nk8 * h], FP8, tag="chi8",
                                       name=f"chi8_{b}")
                nc.sync.dma_start(chi_t[:], chi8_ext[b])
                chb_t = chib_pool.tile([128, nkb16 * h], BF16, tag="chib",
                                       name=f"chib_{b}")
                nc.sync.dma_start(chb_t[:], chib_ext[b])
                return clo_t, (chi_t, chb_t)

            def emit_qk_softmax(b, s, ti, ct_t):
                """Scores for q-tile (s, ti) + softmax chain.

                kbi-outer: each 512-col PSUM bank finishes its hc
                accumulation early, so its partial row-max runs on DVE
                while the next bank's matmuls stream, and each exp
                releases its bank for the next q-tile's QK (per-bank
                tiles from a bufs=4 pool) with ~1.5 us latency instead
                of a 4.4 us whole-tile WAR.

                exp output is the UNNORMALIZED bf16 P (max ~= 1); the
                row-sums of exact exp values accumulate into l_tot and
                the reciprocal is kept for the PV-drain renormalize."""
                t = s * SUPER + ti
                qt_t = qt_map[(b, s)]
                pm = stats_pool.tile([128, n_kb], F32, tag="pm",
                                     name=f"pm_{b}_{t}")
                banks = []
                for kbi in range(n_kb):
                    sb = ps_s.tile([128, kb], F32, tag="s",
                                   name=f"s_{b}_{t}_{kbi}")
                    for hc in range(n_hc):
                        lhs = qt_t[:, hc * sq + ti * 128:
                                   hc * sq + (ti + 1) * 128]
                        rhs = ct_t[:, kbi * n_hc * kb + hc * kb:
                                   kbi * n_hc * kb + (hc + 1) * kb]
                        nc.tensor.matmul(
                            sb[:], lhs, rhs,
                            start=(hc == 0), stop=(hc == n_hc - 1))
                    nc.vector.reduce_max(
                        pm[:, kbi:kbi + 1], sb[:], axis=mybir.AxisListType.X)
                    banks.append(sb)
                negm = stats_pool.tile([128, 1], F32, tag="negm",
                                       name=f"negm_{b}_{t}")
                nc.vector.reduce_max(
                    negm[:], pm[:], axis=mybir.AxisListType.X, negate=True)
                lacc = stats_pool.tile([128, n_kb], F32, tag="lacc",
                                       name=f"lacc_{b}_{t}")
                nc.vector.memset(lacc[:], 0.0)
                p = p_pool.tile([128, tc], BF16, tag="p", name=f"p_{b}_{t}")
                for kbi, sb in enumerate(banks):
                    nc.scalar.activation(
                        p[:, kbi * kb:(kbi + 1) * kb], sb[:],
                        mybir.ActivationFunctionType.Exp,
                        bias=negm[:], scale=1.0,
                        accum_out=lacc[:, kbi:kbi + 1])
                l_tot = stats_pool.tile([128, 1], F32, tag="ltot",
                                        name=f"lt_{b}_{t}")
                nc.vector.reduce_sum(l_tot[:], lacc[:],
                                     axis=mybir.AxisListType.X)
                rcp = stats_pool.tile([128, 1], F32, tag="rcp",
                                      name=f"rcp_{b}_{t}")
                nc.vector.reciprocal(rcp[:], l_tot[:])
                p_tiles[(b, t)] = p
                rcp_tiles[(b, t)] = rcp

            def emit_rcpb_row(b, s):
                """Transpose the SUPER rcp [128,1] columns into one row."""
                row_ps = ps_tp.tile([128, sq], F32, tag="tp",
                                    name=f"rrow_{b}_{s}")
                for ti in range(SUPER):
                    rcp = rcp_tiles.pop((b, s * SUPER + ti))
                    nc.tensor.transpose(
                        row_ps[0:1, ti * 128:(ti + 1) * 128], rcp[:], idf[:])
                row_sb = rrow_pool.tile([1, sq], F32R, tag="rrow",
                                        name=f"rrs_{b}_{s}")
                nc.scalar.copy(row_sb[:], row_ps[0:1, 0:sq])
                return row_sb

            def emit_rcpb_bcast(b, s, row_sb):
                """Ones outer-product broadcast of 1/rowsum to [128, sq]."""
                bc_ps = ps_mm.tile([128, sq], F32, tag="mm",
                                   name=f"rbc_{b}_{s}")
                nc.tensor.matmul(bc_ps[:], ones_r[:], row_sb[:],
                                 start=True, stop=True)
                rcpb = rcpb_pool.tile([128, sq], F32, tag="rcpb",
                                      name=f"rcpb_{b}_{s}")
                nc.vector.tensor_copy(rcpb[:], bc_ps[:])
                rcpb_map[(b, s)] = rcpb

            def emit_pt(b, s):
                """P^T for super s: bf16 PE transposes packed into PSUM
                banks, drained by wide ACT/DVE copies casting each
                k-half to its PV dtype."""
                pt_lo = ptlo_pool.tile([128, nk2 * sq], FP8, tag="ptlo",
                                       name=f"ptlo_{b}_{s}")
                pt_hi = pthi8_pool.tile([128, nk8 * sq], FP8,
                                        tag="pthi8", name=f"pthi_{b}_{s}")
                pt_hb = pthib_pool.tile([128, nkb16 * sq], BF16,
                                        tag="pthib", name=f"pthb_{b}_{s}")
                ps = [p_tiles.pop((b, s * SUPER + ti)) for ti in range(SUPER)]
                # pack one k-tile PAIR x all SUPER q-tiles per PSUM bank so
                # the drain is a single fully-CONTIGUOUS [128, 2*sq] copy
                # into the [k, q] pt layout (strided drains measured 2x
                # slower and made P^T drain-paced).
                for kp in range(n_kt // 2):
                    if kp < nk2 // 2:
                        tgt, kbase = pt_lo, 0
                    elif kp < (nk2 + nk8) // 2:
                        tgt, kbase = pt_hi, nk2
                    else:
                        tgt, kbase = pt_hb, nk2 + nk8
                    tp8 = ps_tp.tile([128, 2 * sq], BF16, tag="tp",
                                     name=f"tp8_{b}_{s}_{kp}")
                    for j in range(2):
                        kt = kp * 2 + j
                        for ti in range(SUPER):
                            nc.tensor.transpose(
                                tp8[:, (j * SUPER + ti) * 128:
                                    (j * SUPER + ti + 1) * 128],
                                ps[ti][:, kt * 128:(kt + 1) * 128],
                                idb[:])
                    dst = tgt[:, (kp * 2 - kbase) * sq:
                              (kp * 2 - kbase + 2) * sq]
                    # bias drains toward ACT: DVE is the PV-phase
                    # bottleneck (reduce_max + renormalize muls)
                    if kp % 4 == 3:
                        nc.vector.tensor_copy(dst, tp8[:])
                    else:
                        nc.scalar.copy(dst, tp8[:])
                pt_map[(b, s)] = (pt_lo, pt_hi, pt_hb)

            def emit_pv(b, s, c_lo, c_hi, hcs):
                """PV matmuls for h-chunks `hcs`: mix^T into combT.

                fp8 DoubleRow over the first nk2+nk8 k-tiles (adjacent
                k-tile pairs), bf16 for the trailing nkb16.  Drain
                multiplies by the rcpb broadcast (renormalize)."""
                combT = combT_map[(b, s)]
                pt_lo, pt_hi, pt_hb = pt_map[(b, s)]
                rcpb = rcpb_map[(b, s)]
                chi_t, chb_t = c_hi
                plo_r = pt_lo.rearrange("p (k q) -> p k q", q=sq)
                phi_r = pt_hi.rearrange("p (k q) -> p k q", q=sq)
                def dr_lhs(c_t, pk, hc):
                    if SW_INTERLEAVE:
                        # [pair, hc, 2x128 interleaved-reversed] layout
                        off = pk * 2 * h + hc * 256
                        return c_t[:, off:off + 256].rearrange(
                            "p (two c) -> p two c", two=2)
                    r = c_t.rearrange("p (k h2) -> p k h2", h2=h)
                    return r[:, 2 * pk:2 * pk + 2, hc * 128:(hc + 1) * 128]

                dr_mode = (mybir.MatmulPerfMode.DoubleRowSwInterleave
                           if SW_INTERLEAVE else
                           mybir.MatmulPerfMode.DoubleRow)
                for hc in hcs:
                    mm = ps_mm.tile([128, sq], F32, tag="mm",
                                    name=f"mm_{b}_{s}_{hc}")
                    for kt in range(0, nk2, 2):
                        nc.tensor.matmul(
                            mm[:],
                            dr_lhs(c_lo, kt // 2, hc),
                            plo_r[:, kt:kt + 2, :],
                            start=(kt == 0), stop=False,
                            perf_mode=dr_mode)
                    for kt in range(0, nk8, 2):
                        nc.tensor.matmul(
                            mm[:],
                            dr_lhs(chi_t, kt // 2, hc),
                            phi_r[:, kt:kt + 2, :],
                            start=False, stop=False,
                            perf_mode=dr_mode)
                    for kt in range(nkb16):
                        nc.tensor.matmul(
                            mm[:],
                            chb_t[:, kt * h + hc * 128:
                                  kt * h + (hc + 1) * 128],
                            pt_hb[:, kt * sq:(kt + 1) * sq],
                            start=False, stop=(kt == nkb16 - 1))
                    nc.vector.tensor_mul(
                        combT[:, hc * sq:(hc + 1) * sq], mm[:], rcpb[:])

            def emit_proj(b, s, tis):
                """Projection + tanh + store for q-tiles `tis` of super s.

                The concat's Q half is read straight from the fp16 Q^T
                tile; the mix half from combT."""
                combT = combT_map[(b, s)]
                qt_t = qt_map[(b, s)]
                for ti in tis:
                    t = s * SUPER + ti
                    ostage = out_pool.tile([128, ho], F32, tag="ostage",
                                           name=f"os_{b}_{t}")
                    for hb in range(n_hob):
                        pr = ps_mm.tile([128, hob], F32, tag="mm",
                                        name=f"pr_{b}_{t}_{hb}")
                        for dc in range(n_dc):
                            if dc < n_hc:
                                lhs = combT[:, dc * sq + ti * 128:
                                            dc * sq + (ti + 1) * 128]
                            else:
                                lhs = qt_t[:, (dc - n_hc) * sq + ti * 128:
                                           (dc - n_hc) * sq + (ti + 1) * 128]
                            nc.tensor.matmul(
                                pr[:], lhs,
                                wt_t[:, dc * ho + hb * hob:
                                     dc * ho + (hb + 1) * hob],
                                start=(dc == 0), stop=(dc == n_dc - 1))
                        nc.scalar.activation(
                            ostage[:, hb * hob:(hb + 1) * hob], pr[:],
                            mybir.ActivationFunctionType.Tanh)
                    nc.sync.dma_start(
                        out_ext[b, t * 128:(t + 1) * 128, :], ostage[:])

            # ------------------------------------------------------------
            # pipelined main program
            # ------------------------------------------------------------
            prefetched = {}   # b -> (ct_t, clo_t, chi_t); s=0 QKs pre-emitted
            for b in range(b_loc):
                if b in prefetched:
                    ct_t, clo_t, chi_t = prefetched.pop(b)
                    s0_done = True
                else:
                    # startup critical path: only qt(0) + ct stripe 0 gate
                    # the first QK matmuls (qt first -- it gates ALL of
                    # them).  clo/chi/wt (6 MB) are deferred into the
                    # first super's emission (not needed until s=1).
                    emit_qt_dma(b, 0)
                    ct_t = emit_ct_dma(b)
                    emit_const_dma()
                    clo_t = chi_t = None
                    s0_done = False

                for s in range(n_s):
                    if s + 1 < n_s:
                        emit_qt_dma(b, s + 1)
                    if s == 0 and s0_done:
                        continue
                    if s > 0:
                        row_sb = emit_rcpb_row(b, s - 1)
                        emit_rcpb_bcast(b, s - 1, row_sb)
                        combT_map[(b, s - 1)] = comb_pool.tile(
                            [128, n_hc * sq], F16, tag="comb",
                            name=f"cb_{b}_{s - 1}")
                    emit_qk_softmax(b, s, 0, ct_t)
                    if s > 0:
                        emit_pt(b, s - 1)
                        emit_pv(b, s - 1, clo_t, chi_t, range(0, 4))
                    emit_qk_softmax(b, s, 1, ct_t)
                    if s == 0 and clo_t is None:
                        clo_t, chi_t = emit_c_dma(b)
                    if s > 0:
                        emit_pv(b, s - 1, clo_t, chi_t, range(4, n_hc))
                    emit_qk_softmax(b, s, 2, ct_t)
                    if s > 0:
                        emit_proj(b, s - 1, [0, 1])
                    emit_qk_softmax(b, s, 3, ct_t)
                    if s == 0:
                        emit_wt_dma()
                    if s > 0:
                        emit_proj(b, s - 1, [2, 3])
                        pt_map.pop((b, s - 1))
                        rcpb_map.pop((b, s - 1))
                        combT_map.pop((b, s - 1))

                # --- batch tail: last super's tail stages, interleaved
                # with the next batch's prefetch DMAs and (for b+1) its
                # first QK blocks ---
                sl = n_s - 1
                row_sb = emit_rcpb_row(b, sl)
                emit_rcpb_bcast(b, sl, row_sb)
                combT_map[(b, sl)] = comb_pool.tile(
                    [128, n_hc * sq], F16, tag="comb", name=f"cb_{b}_{sl}")
                emit_pt(b, sl)
                nb = b + 1
                if nb < b_loc:
                    # WAR-safe: every reader of ct_t/qt(b,*) is emitted
                    ct_next = emit_ct_dma(nb)
                    emit_qt_dma(nb, 0)
                emit_pv(b, sl, clo_t, chi_t, range(n_hc))
                if nb < b_loc:
                    clo_n, chi_n = emit_c_dma(nb)
                    prefetched[nb] = (ct_next, clo_n, chi_n)
                    emit_qk_softmax(nb, 0, 0, ct_next)
                    emit_proj(b, sl, [0])
                    emit_qk_softmax(nb, 0, 1, ct_next)
                    emit_proj(b, sl, [1])
                    emit_qk_softmax(nb, 0, 2, ct_next)
                    emit_proj(b, sl, [2, 3])
                    emit_qk_softmax(nb, 0, 3, ct_next)
                else:
                    emit_proj(b, sl, [0, 1, 2, 3])
                pt_map.pop((b, sl))
                rcpb_map.pop((b, sl))
                combT_map.pop((b, sl))

            # mark the prefetched first-super QKs of the last batch as
            # consumed bookkeeping (handled inside the loop above via
            # p_tiles/rcp_tiles maps)

    nc.compile()
    return nc


_NC_CACHE = {}


def _get_nc(b_loc, tq, tc, h):
    key = (b_loc, tq, tc, h)
    if key not in _NC_CACHE:
        _NC_CACHE[key] = build_bass(b_loc, tq, tc, h)
    return _NC_CACHE[key]


def make_in_maps(query, context, W_attn, n_cores=N_CORES):
    b = query.shape[0]
    b_loc = b // n_cores
    tq, h = query.shape[1], query.shape[2]
    tc = context.shape[1]
    n_s = tq // (SUPER * 128)
    sq = SUPER * 128
    n_hc = h // 128
    n_kb = tc // 512
    nk2 = (tc // 128) // 2
    n_dc = 2 * h // 128
    F8NP = ml_dtypes.float8_e4m3
    BFNP = ml_dtypes.bfloat16

    q = np.ascontiguousarray(query).reshape(n_cores, b_loc, tq, h)
    c = np.ascontiguousarray(context).reshape(n_cores, b_loc, tc, h)

    # qt[i, b, s, p, hc*sq + j] = Q[i, b, s*sq + j, hc*128 + p]
    qt = np.ascontiguousarray(
        q.reshape(n_cores, b_loc, n_s, sq, n_hc, 128)
        .transpose(0, 1, 2, 5, 4, 3)
        .reshape(n_cores, b_loc, n_s, 128, n_hc * sq)
        .astype(np.float16))
    # ct[i, b, kbi, p, hc*512 + j] = C[i, b, kbi*512 + j, hc*128 + p]
    ct = np.ascontiguousarray(
        c.reshape(n_cores, b_loc, n_kb, 512, n_hc, 128)
        .transpose(0, 1, 2, 5, 4, 3)
        .reshape(n_cores, b_loc, n_kb, 128, n_hc * 512)
        .astype(np.float16))
    # c in [k-tile, h] layout: cl[i, b, p, kt*h + j] = C[i, b, kt*128+p, j]
    nk8 = nk2 - N_KT_BF16
    ckh = (c.reshape(n_cores, b_loc, 2 * nk2, 128, h)
           .transpose(0, 1, 3, 2, 4))  # [i, b, 128, 2*nk2, h]

    def fp8_pairs(kt0, kt1):
        """fp8 stationary chunks for k-tile pairs [kt0, kt1)."""
        npair = (kt1 - kt0) // 2
        blk = ckh[:, :, :, kt0:kt1].astype(F8NP)
        if not SW_INTERLEAVE:
            return np.ascontiguousarray(
                blk.reshape(n_cores, b_loc, 128, (kt1 - kt0) * h))
        # [i, b, p, pair, hc, j, ko] with j reversed within each hc
        # chunk and the pair's two k-tiles interleaved: the PE's native
        # DoubleRow weight read order, stored contiguously.
        a = blk.reshape(n_cores, b_loc, 128, npair, 2, n_hc, 128)
        a = a[:, :, :, :, :, :, ::-1]          # reverse j
        a = a.transpose(0, 1, 2, 3, 5, 6, 4)   # [..., pair, hc, j, ko]
        return np.ascontiguousarray(
            a.reshape(n_cores, b_loc, 128, npair * 2 * h))

    clo = fp8_pairs(0, nk2)
    chi8 = fp8_pairs(nk2, nk2 + nk8)
    chib = np.ascontiguousarray(
        ckh[:, :, :, nk2 + nk8:].reshape(n_cores, b_loc, 128,
                                         N_KT_BF16 * h)
        .astype(BFNP))
    # wt[p, dc*h + j] = W_attn[j, dc*128 + p]
    wt = np.ascontiguousarray(
        np.ascontiguousarray(W_attn.T)
        .reshape(n_dc, 128, h).transpose(1, 0, 2)
        .reshape(128, n_dc * h).astype(np.float16))

    idf = np.eye(128, dtype=np.float32)
    idb = np.eye(128).astype(BFNP)
    ones = np.ones((1, 128), dtype=np.float32)

    in_maps = []
    for i in range(n_cores):
        in_maps.append({
            "qt": qt[i],
            "ct": ct[i],
            "clo": clo[i],
            "chi8": chi8[i],
            "chib": chib[i],
            "wt": wt,
            "idf": idf,
            "idb": idb,
            "ones": ones,
        })
    return in_maps


def kernel(query, context, W_attn, _trace=False, _trace_kwargs=None):
    b, tq, h = query.shape
    tc = context.shape[1]
    b_loc = b // N_CORES
    nc = _get_nc(b_loc, tq, tc, h)
    in_maps = make_in_maps(query, context, W_attn)
    res = run_bass_kernel_spmd(
        nc, in_maps, core_ids=list(range(N_CORES)), trace=_trace,
        **(_trace_kwargs or {}))
    out = np.concatenate([res.results[i]["out"] for i in range(N_CORES)],
                         axis=0)
    if _trace:
        return out, res
    return out


# revision 35
# speedup vs baseline: 1.4801x; 1.0091x over previous
"""Trainium2 8-core kernel for batched attention + concat projection.

Reference computation (per batch b):
    scores = Q @ C^T                  [TQ, TC]
    A      = softmax(scores, axis=-1)
    mix    = A @ C                    [TQ, H]
    out    = tanh(concat([mix, Q]) @ W^T)   [TQ, H]

Distribution: pure data-parallel over batch (B=16 across 8 cores, 2
batches per core), W replicated. No collectives needed.

v2 design (vs the f32r baseline):
  - ALL layout work is done on the host: Q^T / C^T ship pre-transposed
    (fp16), C ships pre-quantized (fp8/bf16) in [k, h] tile layout, and
    W^T ships pre-cast fp16.  The device runs zero f32 transposes and
    zero dtype-staging copies (the baseline burned ~135 us of PE time
    on f32 transposes and ~130 us of DVE on staging casts).
  - QK runs in fp16 (numerically indistinguishable from f32 here:
    sim rel-err identical to 4 decimals).  No fp32 matmuls anywhere
    means fp16/bf16 LDWEIGHTS keep FWL and hide behind the matmul
    stream, where the baseline's fp32 QK was LDWEIGHTS-paced
    (289 ns/MM for a 216 ns ideal).
  - SUPER=4 (512 q columns per super-iteration) so the fp8 DoubleRow
    PV matmuls run at free-dim 512, where DR's ~2x rate is not
    LDW-limited (baseline FD=256 measured 157 ns per 2-k-tile pair;
    FD=512 target ~224 ns per pair covering 2x the columns).
  - softmax over free axis k: DVE reduce_max(negate) -> ACT exp with
    per-partition bias, bf16 output (unnormalized, max ~= 1) and f32
    row-sum accumulator -> DVE reciprocal.  Normalization is folded
    into the PV PSUM drain via a [128, sq] broadcast of 1/rowsum
    (PE: transpose rcp columns to a row + ones outer-product).
  - P^T via bf16 PE transposes packed 8-per-PSUM-bank; drains cast to
    the per-half PV dtype (fp8 lo always; hi fp8 on batch 0, bf16 on
    batch 1 -- same error budget as the baseline, measured 1.74e-2
    against the 2e-2 gate).
  - proj reads the concat's Q half straight out of the fp16 Q^T tile
    (no comb assembly for that half) and runs fp16 x fp16 -> tanh.
  - Stages of super s-1 are interleaved between the QK blocks of
    super s so the in-order PE stream always has ready work while the
    softmax chain (DVE reduce_max -> ACT exp) of the current q-tile
    completes; scores PSUM is WAR-recycled per q-tile.
"""

import numpy as np
import ml_dtypes

import concourse.bacc as bacc
import concourse.tile as tile
import concourse.mybir as mybir
from concourse.bass_utils import run_bass_kernel_spmd

F32 = mybir.dt.float32
F32R = mybir.dt.float32r
F16 = mybir.dt.float16
BF16 = mybir.dt.bfloat16
FP8 = mybir.dt.float8e4

N_CORES = 8
B, TQ, TC, H = 16, 2048, 2048, 1024

SUPER = 4              # q-tiles per super-iteration
PG = 8                 # bf16 transposes packed per PSUM bank


# PV error is dominated by the fp8 quantization of C (P-fp8 alone costs
# only ~0.3%): sim err ~= 0.0176 * sqrt(fp8_fraction), HW ~= 1.14x sim.
# Per-LOCAL-batch count of trailing bf16 k-tiles: batch 0 all-fp8,
# batch 1 keeps 2 bf16 k-tiles (f = 15/16): predicted HW rel err
# ~0.0194 against the 2e-2 gate (model validated on 4 configs).
N_KT_BF16 = (0, 2)

# DoubleRowSwInterleave: ship the fp8 C stationary chunks pre-interleaved
# ([A127, B127, A126, B126, ...] per partition row -- the PE's native DR
# weight read order) so LDWEIGHTS walks memory linearly instead of the
# strided two-tile interleave (plain DR LDW measured 162 ns and exposes
# ~47 ns/MM past the 216 ns DR matmul).
SW_INTERLEAVE = True


def build_bass(b_loc, tq, tc, h, n_cores=N_CORES):
    """Build the per-core Bass graph. All cores run the same graph (SPMD)."""
    d = 2 * h
    ho = h
    n_qt = tq // 128       # q tiles
    n_kt = tc // 128       # k tiles
    n_hc = h // 128        # h chunks
    n_dc = d // 128        # d chunks (contraction for proj)
    kb = 512               # QK rhs block
    n_kb = tc // kb
    hob = 512              # proj output block
    n_hob = ho // hob
    assert n_qt % SUPER == 0
    n_s = n_qt // SUPER
    sq = SUPER * 128       # q columns per super-iteration
    nk2 = n_kt // 2

    def nkb16(b):          # trailing k-tiles of the hi half kept bf16
        return N_KT_BF16[b]

    def nk8(b):            # fp8 k-tiles in the hi half
        return nk2 - nkb16(b)

    nkb_max = max(N_KT_BF16)
    assert all((nk2 - v) % 2 == 0 for v in N_KT_BF16)

    nc = bacc.Bacc("TRN2", target_bir_lowering=False, debug=False,
                   num_devices=n_cores)

    # host-prepped inputs (see make_in_maps for layouts)
    qt_ext = nc.declare_dram_parameter("qt", [b_loc, n_s, 128, n_hc * sq],
                                       F16, isOutput=False)
    ct_ext = nc.declare_dram_parameter("ct", [b_loc, n_kb, 128, n_hc * kb],
                                       F16, isOutput=False)
    clo_ext = nc.declare_dram_parameter("clo", [b_loc, 128, nk2 * h], FP8,
                                        isOutput=False)
    chi8_ext = nc.declare_dram_parameter("chi8", [b_loc, 128, nk2 * h], FP8,
                                         isOutput=False)
    chib_ext = nc.declare_dram_parameter("chib", [b_loc, 128, nkb_max * h],
                                         BF16, isOutput=False)
    wt_ext = nc.declare_dram_parameter("wt", [128, n_dc * ho], F16,
                                       isOutput=False)
    idf_ext = nc.declare_dram_parameter("idf", [128, 128], F32, isOutput=False)
    idb_ext = nc.declare_dram_parameter("idb", [128, 128], BF16, isOutput=False)
    ones_ext = nc.declare_dram_parameter("ones", [1, 128], F32R, isOutput=False)
    out_ext = nc.declare_dram_parameter("out", [b_loc, tq, ho], F32,
                                        isOutput=True)

    with tile.TileContext(nc) as tc_:
        with (
            tc_.tile_pool(name="const", bufs=1) as const_pool,
            tc_.tile_pool(name="ct", bufs=1) as ct_pool,
            tc_.tile_pool(name="qt", bufs=3) as qt_pool,
            tc_.tile_pool(name="clo", bufs=1) as clo_pool,
            tc_.tile_pool(name="chi8", bufs=1) as chi8_pool,
            tc_.tile_pool(name="chib", bufs=1) as chib_pool,
            tc_.tile_pool(name="p", bufs=6) as p_pool,
            tc_.tile_pool(name="ptlo", bufs=1) as ptlo_pool,
            tc_.tile_pool(name="pthi8", bufs=1) as pthi8_pool,
            tc_.tile_pool(name="pthib", bufs=1) as pthib_pool,
            tc_.tile_pool(name="comb", bufs=2) as comb_pool,
            tc_.tile_pool(name="ostage", bufs=3) as out_pool,
            tc_.tile_pool(name="stats", bufs=24) as stats_pool,
            tc_.tile_pool(name="rrow", bufs=2) as rrow_pool,
            tc_.tile_pool(name="rcpb", bufs=2) as rcpb_pool,
            tc_.tile_pool(name="ps_s", bufs=4, space="PSUM") as ps_s,
            tc_.tile_pool(name="ps_tp", bufs=2, space="PSUM") as ps_tp,
            tc_.tile_pool(name="ps_mm", bufs=2, space="PSUM") as ps_mm,
        ):
            # --- constants (wt DMA deferred off the startup critical path) ---
            idf = const_pool.tile([128, 128], F32, tag="idf")
            idb = const_pool.tile([128, 128], BF16, tag="idb")
            ones_r = const_pool.tile([1, 128], F32R, tag="ones")
            wt_t = const_pool.tile([128, n_dc * ho], F16, tag="wt")

            def emit_const_dma():
                nc.sync.dma_start(idf[:], idf_ext[:])
                nc.sync.dma_start(idb[:], idb_ext[:])
                nc.sync.dma_start(ones_r[:], ones_ext[:])

            def emit_wt_dma():
                for half in range(2):
                    nc.sync.dma_start(
                        wt_t[:, half * 8 * ho:(half + 1) * 8 * ho],
                        wt_ext[:, half * 8 * ho:(half + 1) * 8 * ho])

            p_tiles = {}      # (b, t) -> unnormalized bf16 P tile
            rcp_tiles = {}    # (b, t) -> [128, 1] reciprocal row sums
            qt_map = {}       # (b, s) -> fp16 Q^T tile (QK lhs + proj Q-half)
            combT_map = {}    # (b, s) -> fp16 mix^T tile
            pt_map = {}       # (b, s) -> (pt_lo, pt_hi)
            rcpb_map = {}     # (b, s) -> [128, sq] broadcast reciprocal

            def emit_qt_dma(b, s):
                qt_t = qt_pool.tile([128, n_hc * sq], F16, tag="qt",
                                    name=f"qt_{b}_{s}")
                nc.sync.dma_start(qt_t[:], qt_ext[b, s])
                qt_map[(b, s)] = qt_t

            def emit_ct_dma(b):
                ct_t = ct_pool.tile([128, n_kb * n_hc * kb], F16, tag="ct",
                                    name=f"ct_{b}")
                for kbi in range(n_kb):
                    nc.sync.dma_start(
                        ct_t[:, kbi * n_hc * kb:(kbi + 1) * n_hc * kb],
                        ct_ext[b, kbi])
                return ct_t

            def emit_c_dma(b):
                clo_t = clo_pool.tile([128, nk2 * h], FP8, tag="clo",
                                      name=f"clo_{b}")
                nc.sync.dma_start(clo_t[:], clo_ext[b])
                chi_t = chi8_pool.tile([128, nk8(b) * h], FP8, tag="chi8",
                                       name=f"chi8_{b}")
                nc.sync.dma_start(chi_t[:], chi8_ext[b, :, 0:nk8(b) * h])
                chb_t = None
                if nkb16(b):
                    chb_t = chib_pool.tile([128, nkb16(b) * h], BF16,
                                           tag="chib", name=f"chib_{b}")
                    nc.sync.dma_start(chb_t[:],
                                      chib_ext[b, :, 0:nkb16(b) * h])
                return clo_t, (chi_t, chb_t)

            def emit_qk_softmax(b, s, ti, ct_t):
                """Scores for q-tile (s, ti) + softmax chain.

                kbi-outer: each 512-col PSUM bank finishes its hc
                accumulation early, so its partial row-max runs on DVE
                while the next bank's matmuls stream, and each exp
                releases its bank for the next q-tile's QK (per-bank
                tiles from a bufs=4 pool) with ~1.5 us latency instead
                of a 4.4 us whole-tile WAR.

                exp output is the UNNORMALIZED bf16 P (max ~= 1); the
                row-sums of exact exp values accumulate into l_tot and
                the reciprocal is kept for the PV-drain renormalize."""
                t = s * SUPER + ti
                qt_t = qt_map[(b, s)]
                pm = stats_pool.tile([128, n_kb], F32, tag="pm",
                                     name=f"pm_{b}_{t}")
                banks = []
                for kbi in range(n_kb):
                    sb = ps_s.tile([128, kb], F32, tag="s",
                                   name=f"s_{b}_{t}_{kbi}")
                    for hc in range(n_hc):
                        lhs = qt_t[:, hc * sq + ti * 128:
                                   hc * sq + (ti + 1) * 128]
                        rhs = ct_t[:, kbi * n_hc * kb + hc * kb:
                                   kbi * n_hc * kb + (hc + 1) * kb]
                        nc.tensor.matmul(
                            sb[:], lhs, rhs,
                            start=(hc == 0), stop=(hc == n_hc - 1))
                    nc.vector.reduce_max(
                        pm[:, kbi:kbi + 1], sb[:], axis=mybir.AxisListType.X)
                    banks.append(sb)
                negm = stats_pool.tile([128, 1], F32, tag="negm",
                                       name=f"negm_{b}_{t}")
                nc.vector.reduce_max(
                    negm[:], pm[:], axis=mybir.AxisListType.X, negate=True)
                lacc = stats_pool.tile([128, n_kb], F32, tag="lacc",
                                       name=f"lacc_{b}_{t}")
                nc.vector.memset(lacc[:], 0.0)
                p = p_pool.tile([128, tc], BF16, tag="p", name=f"p_{b}_{t}")
                for kbi, sb in enumerate(banks):
                    nc.scalar.activation(
                        p[:, kbi * kb:(kbi + 1) * kb], sb[:],
                        mybir.ActivationFunctionType.Exp,
                        bias=negm[:], scale=1.0,
                        accum_out=lacc[:, kbi:kbi + 1])
                l_tot = stats_pool.tile([128, 1], F32, tag="ltot",
                                        name=f"lt_{b}_{t}")
                nc.vector.reduce_sum(l_tot[:], lacc[:],
                                     axis=mybir.AxisListType.X)
                rcp = stats_pool.tile([128, 1], F32, tag="rcp",
                                      name=f"rcp_{b}_{t}")
                nc.vector.reciprocal(rcp[:], l_tot[:])
                p_tiles[(b, t)] = p
                rcp_tiles[(b, t)] = rcp

            def emit_rcpb_row(b, s):
                """Transpose the SUPER rcp [128,1] columns into one row."""
                row_ps = ps_tp.tile([128, sq], F32, tag="tp",
                                    name=f"rrow_{b}_{s}")
                for ti in range(SUPER):
                    rcp = rcp_tiles.pop((b, s * SUPER + ti))
                    nc.tensor.transpose(
                        row_ps[0:1, ti * 128:(ti + 1) * 128], rcp[:], idf[:])
                row_sb = rrow_pool.tile([1, sq], F32R, tag="rrow",
                                        name=f"rrs_{b}_{s}")
                nc.scalar.copy(row_sb[:], row_ps[0:1, 0:sq])
                return row_sb

            def emit_rcpb_bcast(b, s, row_sb):
                """Ones outer-product broadcast of 1/rowsum to [128, sq]."""
                bc_ps = ps_mm.tile([128, sq], F32, tag="mm",
                                   name=f"rbc_{b}_{s}")
                nc.tensor.matmul(bc_ps[:], ones_r[:], row_sb[:],
                                 start=True, stop=True)
                rcpb = rcpb_pool.tile([128, sq], F32, tag="rcpb",
                                      name=f"rcpb_{b}_{s}")
                nc.vector.tensor_copy(rcpb[:], bc_ps[:])
                rcpb_map[(b, s)] = rcpb

            def emit_pt(b, s):
                """P^T for super s: bf16 PE transposes packed into PSUM
                banks, drained by wide ACT/DVE copies casting each
                k-half to its PV dtype."""
                pt_lo = ptlo_pool.tile([128, nk2 * sq], FP8, tag="ptlo",
                                       name=f"ptlo_{b}_{s}")
                pt_hi = pthi8_pool.tile([128, nk8(b) * sq], FP8,
                                        tag="pthi8", name=f"pthi_{b}_{s}")
                pt_hb = None
                if nkb16(b):
                    pt_hb = pthib_pool.tile([128, nkb16(b) * sq], BF16,
                                            tag="pthib", name=f"pthb_{b}_{s}")
                ps = [p_tiles.pop((b, s * SUPER + ti)) for ti in range(SUPER)]
                # pack one k-tile PAIR x all SUPER q-tiles per PSUM bank so
                # the drain is a single fully-CONTIGUOUS [128, 2*sq] copy
                # into the [k, q] pt layout (strided drains measured 2x
                # slower and made P^T drain-paced).
                for kp in range(n_kt // 2):
                    if kp < nk2 // 2:
                        tgt, kbase = pt_lo, 0
                    elif kp < (nk2 + nk8(b)) // 2:
                        tgt, kbase = pt_hi, nk2
                    else:
                        tgt, kbase = pt_hb, nk2 + nk8(b)
                    tp8 = ps_tp.tile([128, 2 * sq], BF16, tag="tp",
                                     name=f"tp8_{b}_{s}_{kp}")
                    for j in range(2):
                        kt = kp * 2 + j
                        for ti in range(SUPER):
                            nc.tensor.transpose(
                                tp8[:, (j * SUPER + ti) * 128:
                                    (j * SUPER + ti + 1) * 128],
                                ps[ti][:, kt * 128:(kt + 1) * 128],
                                idb[:])
                    dst = tgt[:, (kp * 2 - kbase) * sq:
                              (kp * 2 - kbase + 2) * sq]
                    # bias drains toward ACT: DVE is the PV-phase
                    # bottleneck (reduce_max + renormalize muls)
                    if kp % 4 == 3:
                        nc.vector.tensor_copy(dst, tp8[:])
                    else:
                        nc.scalar.copy(dst, tp8[:])
                pt_map[(b, s)] = (pt_lo, pt_hi, pt_hb)

            def emit_pv(b, s, c_lo, c_hi, hcs):
                """PV matmuls for h-chunks `hcs`: mix^T into combT.

                fp8 DoubleRow over the first nk2+nk8 k-tiles (adjacent
                k-tile pairs), bf16 for the trailing nkb16.  Drain
                multiplies by the rcpb broadcast (renormalize)."""
                combT = combT_map[(b, s)]
                pt_lo, pt_hi, pt_hb = pt_map[(b, s)]
                rcpb = rcpb_map[(b, s)]
                chi_t, chb_t = c_hi
                plo_r = pt_lo.rearrange("p (k q) -> p k q", q=sq)
                phi_r = pt_hi.rearrange("p (k q) -> p k q", q=sq)
                def dr_lhs(c_t, pk, hc):
                    if SW_INTERLEAVE:
                        # [pair, hc, 2x128 interleaved-reversed] layout
                        off = pk * 2 * h + hc * 256
                        return c_t[:, off:off + 256].rearrange(
                            "p (two c) -> p two c", two=2)
                    r = c_t.rearrange("p (k h2) -> p k h2", h2=h)
                    return r[:, 2 * pk:2 * pk + 2, hc * 128:(hc + 1) * 128]

                dr_mode = (mybir.MatmulPerfMode.DoubleRowSwInterleave
                           if SW_INTERLEAVE else
                           mybir.MatmulPerfMode.DoubleRow)
                for hc in hcs:
                    mm = ps_mm.tile([128, sq], F32, tag="mm",
                                    name=f"mm_{b}_{s}_{hc}")
                    for kt in range(0, nk2, 2):
                        nc.tensor.matmul(
                            mm[:],
                            dr_lhs(c_lo, kt // 2, hc),
                            plo_r[:, kt:kt + 2, :],
                            start=(kt == 0), stop=False,
                            perf_mode=dr_mode)
                    nhi = nk8(b)
                    for kt in range(0, nhi, 2):
                        nc.tensor.matmul(
                            mm[:],
                            dr_lhs(chi_t, kt // 2, hc),
                            phi_r[:, kt:kt + 2, :],
                            start=False, stop=(nkb16(b) == 0
                                               and kt == nhi - 2),
                            perf_mode=dr_mode)
                    for kt in range(nkb16(b)):
                        nc.tensor.matmul(
                            mm[:],
                            chb_t[:, kt * h + hc * 128:
                                  kt * h + (hc + 1) * 128],
                            pt_hb[:, kt * sq:(kt + 1) * sq],
                            start=False, stop=(kt == nkb16(b) - 1))
                    # the renormalize mul gates mm-bank reuse (ps_mm
                    # bufs=2): schedule it ahead of the next q-tile's
                    # DVE reduces so the PE's DR stream isn't WAR-stalled
                    with tc_.high_priority(offset=64):
                        nc.vector.tensor_mul(
                            combT[:, hc * sq:(hc + 1) * sq], mm[:], rcpb[:])

            def emit_proj(b, s, tis):
                """Projection + tanh + store for q-tiles `tis` of super s.

                The concat's Q half is read straight from the fp16 Q^T
                tile; the mix half from combT."""
                combT = combT_map[(b, s)]
                qt_t = qt_map[(b, s)]
                for ti in tis:
                    t = s * SUPER + ti
                    ostage = out_pool.tile([128, ho], F32, tag="ostage",
                                           name=f"os_{b}_{t}")
                    for hb in range(n_hob):
                        pr = ps_mm.tile([128, hob], F32, tag="mm",
                                        name=f"pr_{b}_{t}_{hb}")
                        for dc in range(n_dc):
                            if dc < n_hc:
                                lhs = combT[:, dc * sq + ti * 128:
                                            dc * sq + (ti + 1) * 128]
                            else:
                                lhs = qt_t[:, (dc - n_hc) * sq + ti * 128:
                                           (dc - n_hc) * sq + (ti + 1) * 128]
                            nc.tensor.matmul(
                                pr[:], lhs,
                                wt_t[:, dc * ho + hb * hob:
                                     dc * ho + (hb + 1) * hob],
                                start=(dc == 0), stop=(dc == n_dc - 1))
                        nc.scalar.activation(
                            ostage[:, hb * hob:(hb + 1) * hob], pr[:],
                            mybir.ActivationFunctionType.Tanh)
                    nc.sync.dma_start(
                        out_ext[b, t * 128:(t + 1) * 128, :], ostage[:])

            # ------------------------------------------------------------
            # pipelined main program
            # ------------------------------------------------------------
            prefetched = {}   # b -> (ct_t, clo_t, chi_t); s=0 QKs pre-emitted
            for b in range(b_loc):
                if b in prefetched:
                    ct_t, clo_t, chi_t = prefetched.pop(b)
                    s0_done = True
                else:
                    # startup critical path: only qt(0) + ct stripe 0 gate
                    # the first QK matmuls (qt first -- it gates ALL of
                    # them).  clo/chi/wt (6 MB) are deferred into the
                    # first super's emission (not needed until s=1).
                    emit_qt_dma(b, 0)
                    ct_t = emit_ct_dma(b)
                    emit_const_dma()
                    clo_t = chi_t = None
                    s0_done = False

                for s in range(n_s):
                    if s + 1 < n_s:
                        emit_qt_dma(b, s + 1)
                    if s == 0 and s0_done:
                        continue
                    if s > 0:
                        row_sb = emit_rcpb_row(b, s - 1)
                        emit_rcpb_bcast(b, s - 1, row_sb)
                        combT_map[(b, s - 1)] = comb_pool.tile(
                            [128, n_hc * sq], F16, tag="comb",
                            name=f"cb_{b}_{s - 1}")
                    emit_qk_softmax(b, s, 0, ct_t)
                    if s > 0:
                        emit_pt(b, s - 1)
                        emit_pv(b, s - 1, clo_t, chi_t, range(0, 4))
                    emit_qk_softmax(b, s, 1, ct_t)
                    if s == 0 and clo_t is None:
                        clo_t, chi_t = emit_c_dma(b)
                    if s > 0:
                        emit_pv(b, s - 1, clo_t, chi_t, range(4, n_hc))
                    emit_qk_softmax(b, s, 2, ct_t)
                    if s > 0:
                        emit_proj(b, s - 1, [0, 1])
                    emit_qk_softmax(b, s, 3, ct_t)
                    if s == 0:
                        emit_wt_dma()
                    if s > 0:
                        emit_proj(b, s - 1, [2, 3])
                        pt_map.pop((b, s - 1))
                        rcpb_map.pop((b, s - 1))
                        combT_map.pop((b, s - 1))

                # --- batch tail: last super's tail stages, interleaved
                # with the next batch's prefetch DMAs and (for b+1) its
                # first QK blocks ---
                sl = n_s - 1
                row_sb = emit_rcpb_row(b, sl)
                emit_rcpb_bcast(b, sl, row_sb)
                combT_map[(b, sl)] = comb_pool.tile(
                    [128, n_hc * sq], F16, tag="comb", name=f"cb_{b}_{sl}")
                emit_pt(b, sl)
                nb = b + 1
                if nb < b_loc:
                    # WAR-safe: every reader of ct_t/qt(b,*) is emitted
                    ct_next = emit_ct_dma(nb)
                    emit_qt_dma(nb, 0)
                emit_pv(b, sl, clo_t, chi_t, range(n_hc))
                if nb < b_loc:
                    clo_n, chi_n = emit_c_dma(nb)
                    prefetched[nb] = (ct_next, clo_n, chi_n)
                    emit_qk_softmax(nb, 0, 0, ct_next)
                    emit_proj(b, sl, [0])
                    emit_qk_softmax(nb, 0, 1, ct_next)
                    emit_proj(b, sl, [1])
                    emit_qk_softmax(nb, 0, 2, ct_next)
                    emit_proj(b, sl, [2, 3])
                    emit_qk_softmax(nb, 0, 3, ct_next)
                else:
                    emit_proj(b, sl, [0, 1, 2, 3])
                pt_map.pop((b, sl))
                rcpb_map.pop((b, sl))
                combT_map.pop((b, sl))

            # mark the prefetched first-super QKs of the last batch as
            # consumed bookkeeping (handled inside the loop above via
            # p_tiles/rcp_tiles maps)

    nc.compile()
    return nc


_NC_CACHE = {}


def _get_nc(b_loc, tq, tc, h):
    key = (b_loc, tq, tc, h)
    if key not in _NC_CACHE:
        _NC_CACHE[key] = build_bass(b_loc, tq, tc, h)
    return _NC_CACHE[key]


def make_in_maps(query, context, W_attn, n_cores=N_CORES):
    b = query.shape[0]
    b_loc = b // n_cores
    tq, h = query.shape[1], query.shape[2]
    tc = context.shape[1]
    n_s = tq // (SUPER * 128)
    sq = SUPER * 128
    n_hc = h // 128
    n_kb = tc // 512
    nk2 = (tc // 128) // 2
    n_dc = 2 * h // 128
    F8NP = ml_dtypes.float8_e4m3
    BFNP = ml_dtypes.bfloat16

    q = np.ascontiguousarray(query).reshape(n_cores, b_loc, tq, h)
    c = np.ascontiguousarray(context).reshape(n_cores, b_loc, tc, h)

    # qt[i, b, s, p, hc*sq + j] = Q[i, b, s*sq + j, hc*128 + p]
    qt = np.ascontiguousarray(
        q.reshape(n_cores, b_loc, n_s, sq, n_hc, 128)
        .transpose(0, 1, 2, 5, 4, 3)
        .reshape(n_cores, b_loc, n_s, 128, n_hc * sq)
        .astype(np.float16))
    # ct[i, b, kbi, p, hc*512 + j] = C[i, b, kbi*512 + j, hc*128 + p]
    ct = np.ascontiguousarray(
        c.reshape(n_cores, b_loc, n_kb, 512, n_hc, 128)
        .transpose(0, 1, 2, 5, 4, 3)
        .reshape(n_cores, b_loc, n_kb, 128, n_hc * 512)
        .astype(np.float16))
    # c in [k-tile, h] layout: cl[i, b, p, kt*h + j] = C[i, b, kt*128+p, j]
    nkb_max = max(N_KT_BF16)
    ckh = (c.reshape(n_cores, b_loc, 2 * nk2, 128, h)
           .transpose(0, 1, 3, 2, 4))  # [i, b, 128, 2*nk2, h]

    def fp8_pairs(kt0, kt1):
        """fp8 stationary chunks for k-tile pairs [kt0, kt1)."""
        npair = (kt1 - kt0) // 2
        blk = ckh[:, :, :, kt0:kt1].astype(F8NP)
        if not SW_INTERLEAVE:
            return np.ascontiguousarray(
                blk.reshape(n_cores, b_loc, 128, (kt1 - kt0) * h))
        # [i, b, p, pair, hc, j, ko] with j reversed within each hc
        # chunk and the pair's two k-tiles interleaved: the PE's native
        # DoubleRow weight read order, stored contiguously.
        a = blk.reshape(n_cores, b_loc, 128, npair, 2, n_hc, 128)
        a = a[:, :, :, :, :, :, ::-1]          # reverse j
        a = a.transpose(0, 1, 2, 3, 5, 6, 4)   # [..., pair, hc, j, ko]
        return np.ascontiguousarray(
            a.reshape(n_cores, b_loc, 128, npair * 2 * h))

    # chi8 covers the FULL hi half as fp8 pairs; chib the trailing
    # nkb_max k-tiles as bf16.  Each batch's graph reads only the
    # prefix/suffix its N_KT_BF16[b] config needs.
    clo = fp8_pairs(0, nk2)
    chi8 = fp8_pairs(nk2, 2 * nk2)
    chib = np.ascontiguousarray(
        ckh[:, :, :, 2 * nk2 - nkb_max:].reshape(n_cores, b_loc, 128,
                                                 nkb_max * h)
        .astype(BFNP))
    # wt[p, dc*h + j] = W_attn[j, dc*128 + p]
    wt = np.ascontiguousarray(
        np.ascontiguousarray(W_attn.T)
        .reshape(n_dc, 128, h).transpose(1, 0, 2)
        .reshape(128, n_dc * h).astype(np.float16))

    idf = np.eye(128, dtype=np.float32)
    idb = np.eye(128).astype(BFNP)
    ones = np.ones((1, 128), dtype=np.float32)

    in_maps = []
    for i in range(n_cores):
        in_maps.append({
            "qt": qt[i],
            "ct": ct[i],
            "clo": clo[i],
            "chi8": chi8[i],
            "chib": chib[i],
            "wt": wt,
            "idf": idf,
            "idb": idb,
            "ones": ones,
        })
    return in_maps


def kernel(query, context, W_attn, _trace=False, _trace_kwargs=None):
    b, tq, h = query.shape
    tc = context.shape[1]
    b_loc = b // N_CORES
    nc = _get_nc(b_loc, tq, tc, h)
    in_maps = make_in_maps(query, context, W_attn)
    res = run_bass_kernel_spmd(
        nc, in_maps, core_ids=list(range(N_CORES)), trace=_trace,
        **(_trace_kwargs or {}))
    out = np.concatenate([res.results[i]["out"] for i in range(N_CORES)],
                         axis=0)
    if _trace:
        return out, res
    return out
